# revision 12
# baseline (speedup 1.0000x reference)
"""Trainium2 Bass kernel for nn_ExpaModel_73478300500036 (3-layer GATv2-style
GNN message passing, N=16384 nodes, E=49152 edges, D=768, H=4 heads).

Strategy (8 NeuronCores, SPMD, dst-sharded):
  - core k owns nodes [k*2048, (k+1)*2048) and computes their output rows.
  - Per layer: hs = x @ Wsrc computed for the own shard, AllGathered to all
    cores (the only collective); hd, he stay local.
  - Real edges grouped by 128-node dst windows, 128 edges/chunk.  Per chunk:
    one indirect row-gather of hs[src]; hd[dst] and he[attr] come from
    one-hot permutation matmuls on the PE (no DMA gathers):
      u = hsg + eqT_dst.T @ hd_win + eqT_attr.T @ he_table
    z = leaky(u), logits via per-head multiply+reduce, p = exp(logits)
    (segment max skipped: logits are O(10), exp is safe in fp32 and
    softmax is shift-invariant).
  - Scatter via one-hot matmuls: po += eq.T @ (p_h * hsg), den via rhs=p.
  - Self-loops (attr = mean of incoming) handled densely per window:
    z_self = leaky(hs_win + hd_win + Cdiv @ he_table); their scatter
    contribution enters the same PSUM groups via lhsT=identity.
  - PSUM: a single rotating pair of [128,2048] fp32 slots (4 banks each)
    serves weight matmuls, permute accumulation, and scatter+den.
  - Projection + LayerNorm + gelu data-parallel over nodes.

Storage dtype bf16, fp32 accumulation.
"""

import os
import sys

sys.path.insert(0, "/opt/trn_rl_repo")

# The RDH collective algorithm (1-56MB messages) crashes the device in this
# environment; force mesh/ring instead.
os.environ.setdefault("NEURON_RT_DBG_RDH_CC", "0")

import ml_dtypes
import numpy as np

import concourse.bass as bass
import concourse.mybir as mybir
import concourse.tile as tile
from concourse.bass_utils import run_bass_kernel_spmd
from concourse.masks import make_identity
from concourse.vector_clock import ScopedClock

# ----------------------------------------------------------------------------
# Workaround: this container's walrus build supports at most ONE sync wait per
# instruction. (a) Tile's tail drain carries several waits -> emit them as
# separate SP EventSemaphore waits; (b) post-pass splits any remaining
# multi-wait instruction.
# ----------------------------------------------------------------------------


def _patched_drain_and_barrier(self, tick_clock, wait_clock):
    nc = self.nc
    probe = mybir.InstDrain(
        name=nc.get_next_instruction_name(), ins=[], outs=[], bass_is_fusable=False
    )
    probe.engine = mybir.EngineType.SP
    wait_clock.add_sem_waits(probe, ScopedClock({None: tick_clock.global_clock}))
    waits = []
    si = probe.sync_info
    if si is not None and si.on_wait:
        waits = list(si.on_wait)
    id2sem = {h.num: h for h in self.sems.allocated().values()}
    for w in waits:
        sem = id2sem.get(w.id)
        assert sem is not None, f"drain wait on unknown sem id {w.id}"
        nc.sync.wait_ge(sem, w.wait_value)
    nc.sync.drain()
    nc.all_engine_barrier()
    assert self.sems is not None
    popped = nc._tile_sem_poison_stack.pop()
    assert popped is self._sem_poison
    nc.clear_and_free_semaphores(list(self.sems.allocated().values()))
    nc.all_engine_barrier()


tile.TileContext._drain_and_barrier = _patched_drain_and_barrier

_split_n = [0]


def _split_multi_waits(nc):
    for f in nc.m.functions:
        for bb in f.blocks:
            insts = list(bb.instructions)
            changed = False
            new_list = []
            for inst in insts:
                si = inst.sync_info
                waits = list(si.on_wait) if (si is not None and si.on_wait) else []
                if len(waits) > 1:
                    changed = True
                    for w in waits[:-1]:
                        _split_n[0] += 1
                        ev = mybir.InstEventSemaphore(
                            name=f"evsplit-{_split_n[0]}", ins=[], outs=[]
                        )
                        ev.engine = inst.engine
                        ev.sync_info = mybir.SyncInfo(on_wait=[w], on_update=[])
                        new_list.append(ev)
                    inst.sync_info = mybir.SyncInfo(
                        on_wait=[waits[-1]],
                        on_update=list(si.on_update) if si.on_update else [],
                    )
                new_list.append(inst)
            if changed:
                bb.instructions = new_list


# ----------------------------------------------------------------------------
# Problem constants (hardcoded per spec)
# ----------------------------------------------------------------------------
NCORES = 8
N = 16384
E = 49152
D = 768
H = 4
L = 3
R = 64
HD = H * D  # 3072
HH = HD // 2  # 1536 (head-pair half)
NL = N // NCORES  # 2048
NW = NL // 128  # 16 windows per core
WSZ = 128
LN_EPS = 1e-5
NEG_SLOPE = 0.2

F32 = mybir.dt.float32
BF16 = mybir.dt.bfloat16
I32 = mybir.dt.int32

AF = mybir.ActivationFunctionType
ALU = mybir.AluOpType


def build_program(cpw, nch):
    nc = bass.Bass("TRN2", num_devices=NCORES)
    maxc = max(cpw)

    # ---- I/O ----
    x_in = nc.dram_tensor("x_own", [NL, D], F32, kind="ExternalInput")
    xT0_in = nc.dram_tensor("xT0", [128, 6 * NL], BF16, kind="ExternalInput")
    idx_in = nc.dram_tensor("idx", [128, nch * 2], I32, kind="ExternalInput")
    rep_in = nc.dram_tensor("rep2", [128, nch * 256], BF16, kind="ExternalInput")
    cdt_in = nc.dram_tensor("cdivT", [R, NL], BF16, kind="ExternalInput")
    ws_in = nc.dram_tensor("w_src", [L, D, HD], BF16, kind="ExternalInput")
    wd_in = nc.dram_tensor("w_dst", [L, D, HD], BF16, kind="ExternalInput")
    we_in = nc.dram_tensor("w_edge", [L, D, HD], BF16, kind="ExternalInput")
    rel_in = nc.dram_tensor("rel_emb", [R, D], F32, kind="ExternalInput")
    att_in = nc.dram_tensor("att_rep", [L, 128, HD], BF16, kind="ExternalInput")
    bias_in = nc.dram_tensor("bias_rep", [L, 128, D], F32, kind="ExternalInput")
    pw_in = nc.dram_tensor("proj_w", [D, D], BF16, kind="ExternalInput")
    pb_in = nc.dram_tensor("pb_rep", [128, D], BF16, kind="ExternalInput")
    lng_in = nc.dram_tensor("lng_rep", [128, D], BF16, kind="ExternalInput")
    lnb_in = nc.dram_tensor("lnb_rep", [128, D], BF16, kind="ExternalInput")
    out_t = nc.dram_tensor("out", [NL, D], F32, kind="ExternalOutput")

    # ---- internal DRAM ----
    ag_in = nc.dram_tensor("ag_in", [NL, HD], BF16, kind="Internal")
    hs_full = nc.dram_tensor(
        "hs_full", [N, HD], BF16, kind="Internal", addr_space="Shared"
    )
    hd_dram = nc.dram_tensor("hd_dram", [NL, HD], BF16, kind="Internal")
    xb = [nc.dram_tensor(f"xb{i}", [NL, D], F32, kind="Internal") for i in range(2)]

    with tile.TileContext(nc) as tc:
        with (
            tc.tile_pool(name="sb", bufs=1) as sp,
            tc.tile_pool(name="ps", bufs=1, space="PSUM") as pp,
        ):
            # ---- static tiles ----
            ident = sp.tile([128, 128], F32, tag="ident")
            make_identity(nc, ident[:])
            identb = sp.tile([128, 128], BF16, tag="identb")
            nc.vector.tensor_copy(out=identb[:], in_=ident[:])
            ioti = sp.tile([128, 128], mybir.dt.int16, tag="ioti")
            nc.gpsimd.iota(ioti[:], pattern=[[1, 128]], base=0, channel_multiplier=0)
            iotab = sp.tile([128, 128], BF16, tag="iotab")
            nc.vector.tensor_copy(out=iotab[:], in_=ioti[:])
            ioPi = sp.tile([128, 1], I32, tag="ioPi")
            nc.gpsimd.iota(ioPi[:], pattern=[[0, 1]], base=0, channel_multiplier=1)
            ioPb = sp.tile([128, 1], BF16, tag="ioPb")
            nc.vector.tensor_copy(out=ioPb[:], in_=ioPi[:])
            ones_b = sp.tile([128, 1], BF16, tag="ones")
            nc.vector.memset(ones_b[:], 1.0)

            idx_t = sp.tile([128, nch * 2], I32, tag="idx")
            nc.sync.dma_start(out=idx_t[:], in_=idx_in[:])
            idx2 = idx_t[:].rearrange("p (c f) -> p c f", f=2)

            cdt = sp.tile([R, NL], BF16, tag="cdt")
            nc.sync.dma_start(out=cdt[:], in_=cdt_in[:])
            rel_sb = sp.tile([R, D], F32, tag="rel")
            nc.sync.dma_start(out=rel_sb[:], in_=rel_in[:])

            # relT [768, 64] as 6 blocks [128, 64], via PE transpose (fp32)
            relT = sp.tile([128, 6 * R], BF16, tag="relT")
            for kt in range(6):
                pt = pp.tile([128, 2048], F32, tag="psA", bufs=2)
                nc.tensor.transpose(
                    out=pt[:, 0:R],
                    in_=rel_sb[:, kt * 128 : (kt + 1) * 128],
                    identity=ident[:R, :R],
                )
                nc.scalar.copy(out=relT[:, kt * R : (kt + 1) * R], in_=pt[:, 0:R])

            pbb = sp.tile([128, D], BF16, tag="pbb")
            nc.sync.dma_start(out=pbb[:], in_=pb_in[:])
            lngb = sp.tile([128, D], BF16, tag="lngb")
            nc.sync.dma_start(out=lngb[:], in_=lng_in[:])
            lnbb = sp.tile([128, D], BF16, tag="lnbb")
            nc.sync.dma_start(out=lnbb[:], in_=lnb_in[:])

            het = sp.tile([R, HD], BF16, tag="het")
            pself_all = sp.tile([128, 4 * NW], F32, tag="pself")

            def wload(w_dram, l, fh):
                wt = sp.tile([128, 6 * 1536], BF16, tag="wt", bufs=2)
                for kt in range(6):
                    nc.sync.dma_start(
                        out=wt[:, kt * 1536 : (kt + 1) * 1536],
                        in_=w_dram[
                            l, kt * 128 : (kt + 1) * 128, fh * 1536 : (fh + 1) * 1536
                        ],
                    )
                return wt

            def proj_half(xT, wt, fh, dest):
                """dest[:, fh half] <- x @ W (one 1536-col half)."""
                for m in range(NW):
                    ps = pp.tile([128, 2048], F32, tag="psA", bufs=2)
                    for kt in range(6):
                        for s in range(3):
                            nc.tensor.matmul(
                                ps[:, s * 512 : (s + 1) * 512],
                                lhsT=xT[:, kt * NL + m * 128 : kt * NL + (m + 1) * 128],
                                rhs=wt[:, kt * 1536 + s * 512 : kt * 1536 + (s + 1) * 512],
                                start=(kt == 0),
                                stop=(kt == 5),
                            )
                    ev = sp.tile([128, 1536], BF16, tag="stage", bufs=2)
                    nc.scalar.copy(out=ev[:], in_=ps[:, 0:1536])
                    nc.sync.dma_start(
                        out=dest[m * 128 : (m + 1) * 128, fh * 1536 : (fh + 1) * 1536],
                        in_=ev[:],
                    )

            x_cur = x_in
            xT = None
            for l in range(L):
                if l == 0:
                    # ---- xT + hs -> ag_in (layers >0: done inside edge(l-1)) ----
                    xT = sp.tile([128, 6 * NL], BF16, tag="xT")
                    nc.sync.dma_start(out=xT[:], in_=xT0_in[:])
                    for fh in range(2):
                        wt = wload(ws_in, l, fh)
                        proj_half(xT, wt, fh, ag_in)

                att_t = sp.tile([128, HD], BF16, tag="att", bufs=1)
                nc.sync.dma_start(out=att_t[:], in_=att_in[l])
                bias_b = sp.tile([128, D], F32, tag="biasb", bufs=1)
                nc.sync.dma_start(out=bias_b[:], in_=bias_in[l])
                # ---- hd -> hd_dram (overlaps AllGather) ----
                for fh in range(2):
                    wt = wload(wd_in, l, fh)
                    proj_half(xT, wt, fh, hd_dram)

                # ---- he table: het = rel_emb @ Wedge  [64, 3072] ----
                for fh in range(2):
                    wt = wload(we_in, l, fh)
                    ps = pp.tile([128, 2048], F32, tag="psA", bufs=2)
                    for kt in range(6):
                        for s in range(3):
                            nc.tensor.matmul(
                                ps[:R, s * 512 : (s + 1) * 512],
                                lhsT=relT[:, kt * R : (kt + 1) * R],
                                rhs=wt[:, kt * 1536 + s * 512 : kt * 1536 + (s + 1) * 512],
                                start=(kt == 0),
                                stop=(kt == 5),
                            )
                    nc.scalar.copy(
                        out=het[:, fh * 1536 : (fh + 1) * 1536], in_=ps[:R, 0:1536]
                    )

                # ---- self-loop pass per window (overlaps AllGather) ----
                for w in range(NW):
                    rows = slice(w * 128, (w + 1) * 128)
                    slog = sp.tile([128, H], F32, tag="slog", bufs=2)
                    for hp in range(2):
                        csl = slice(hp * 1536, (hp + 1) * 1536)
                        hsw = sp.tile([128, 1536], BF16, tag="hswA", bufs=2)
                        nc.sync.dma_start(out=hsw[:], in_=ag_in[rows, csl])
                        hdw = sp.tile([128, 1536], BF16, tag="hdwA", bufs=2)
                        nc.sync.dma_start(out=hdw[:], in_=hd_dram[rows, csl])
                        ps = pp.tile([128, 2048], F32, tag="psA", bufs=2)
                        for s in range(3):
                            nc.tensor.matmul(
                                ps[:, s * 512 : (s + 1) * 512],
                                lhsT=cdt[:, w * 128 : (w + 1) * 128],
                                rhs=het[:, hp * 1536 + s * 512 : hp * 1536 + (s + 1) * 512],
                                start=True,
                                stop=True,
                            )
                        hl = sp.tile([128, 1536], BF16, tag="uhe", bufs=2)
                        nc.scalar.copy(out=hl[:], in_=ps[:, 0:1536])
                        nc.vector.tensor_add(out=hl[:], in0=hl[:], in1=hsw[:])
                        nc.vector.tensor_add(out=hl[:], in0=hl[:], in1=hdw[:])
                        z = sp.tile([128, 1536], BF16, tag="z", bufs=2)
                        nc.scalar.activation(
                            out=z[:], in_=hl[:], func=AF.Prelu, alpha=NEG_SLOPE
                        )
                        za = sp.tile([128, 1536], BF16, tag="za", bufs=1)
                        nc.vector.tensor_mul(
                            out=za[:], in0=z[:], in1=att_t[:, hp * 1536 : (hp + 1) * 1536]
                        )
                        for hh in range(2):
                            h = hp * 2 + hh
                            nc.vector.tensor_reduce(
                                out=slog[:, h : h + 1],
                                in_=za[:, hh * D : (hh + 1) * D],
                                axis=mybir.AxisListType.X,
                                op=ALU.add,
                            )
                    nc.scalar.activation(
                        out=pself_all[:, 4 * w : 4 * w + 4], in_=slog[:], func=AF.Exp
                    )

                nc.gpsimd.collective_compute(
                    "AllGather",
                    ALU.bypass,
                    ins=[ag_in[:]],
                    outs=[hs_full[:]],
                    replica_groups=[list(range(NCORES))],
                )

                # ---- edge phase (also builds xT(l+1) and hs(l+1) -> ag_in) ----
                xT_nxt = sp.tile([128, 6 * NL], BF16, tag="xT")
                if l < L - 1:
                    wsn = [wload(ws_in, l + 1, fh) for fh in range(2)]
                ci = 0
                for w in range(NW):
                    rows = slice(w * 128, (w + 1) * 128)
                    nch_w = cpw[w]
                    hsw2 = sp.tile([128, HD], BF16, tag="hswB", bufs=2)
                    nc.sync.dma_start(out=hsw2[:], in_=ag_in[rows, :])
                    hdw2 = sp.tile([128, HD], BF16, tag="hdwB", bufs=2)
                    nc.sync.dma_start(out=hdw2[:], in_=hd_dram[rows, :])
                    xc = sp.tile([128, D], F32, tag="xc", bufs=1)
                    nc.sync.dma_start(out=xc[:], in_=x_cur[rows, :])
                    rep_t = sp.tile([128, maxc * 256], BF16, tag="rep", bufs=1)
                    nc.sync.dma_start(
                        out=rep_t[:, 0 : nch_w * 256],
                        in_=rep_in[:, ci * 256 : (ci + nch_w) * 256],
                    )

                    hsgs = []
                    eqs = []
                    ps_list = []
                    logit = sp.tile([128, H * maxc], F32, tag="logit", bufs=2)
                    for c in range(nch_w):
                        cidx = ci + c
                        hsg = sp.tile([128, HD], BF16, tag="hsg", bufs=maxc + 1)
                        nc.gpsimd.indirect_dma_start(
                            out=hsg[:],
                            out_offset=None,
                            in_=hs_full[:, :],
                            in_offset=bass.IndirectOffsetOnAxis(
                                ap=idx2[:, cidx, 0:1], axis=0
                            ),
                        )
                        hsgs.append(hsg)
                        dstf = sp.tile([128, 1], BF16, tag="dstf", bufs=3)
                        nc.vector.tensor_copy(out=dstf[:], in_=idx2[:, cidx, 1:2])
                        eq = sp.tile([128, 128], BF16, tag="eq", bufs=maxc + 2)
                        nc.vector.tensor_tensor(
                            out=eq[:],
                            in0=dstf[:].to_broadcast([128, 128]),
                            in1=iotab[:],
                            op=ALU.is_equal,
                        )
                        eqs.append(eq)
                        eqT = sp.tile([128, 128], BF16, tag="eqT", bufs=3)
                        nc.vector.tensor_tensor(
                            out=eqT[:],
                            in0=ioPb[:].to_broadcast([128, 128]),
                            in1=rep_t[:, c * 256 : c * 256 + 128],
                            op=ALU.is_equal,
                        )
                        eqh = sp.tile([128, 128], BF16, tag="eqh", bufs=3)
                        nc.vector.tensor_tensor(
                            out=eqh[:R, :],
                            in0=ioPb[:R, :].to_broadcast([R, 128]),
                            in1=rep_t[:R, c * 256 + 128 : c * 256 + 256],
                            op=ALU.is_equal,
                        )
                        for hp in range(2):
                            ps = pp.tile([128, 2048], F32, tag="psA", bufs=2)
                            for s in range(3):
                                nc.tensor.matmul(
                                    ps[:, s * 512 : (s + 1) * 512],
                                    lhsT=eqT[:],
                                    rhs=hdw2[
                                        :, hp * 1536 + s * 512 : hp * 1536 + (s + 1) * 512
                                    ],
                                    start=True,
                                    stop=False,
                                )
                            for s in range(3):
                                nc.tensor.matmul(
                                    ps[:, s * 512 : (s + 1) * 512],
                                    lhsT=eqh[:R, :],
                                    rhs=het[
                                        :, hp * 1536 + s * 512 : hp * 1536 + (s + 1) * 512
                                    ],
                                    start=False,
                                    stop=True,
                                )
                            uhe = sp.tile([128, 1536], BF16, tag="uhe", bufs=2)
                            nc.scalar.copy(out=uhe[:], in_=ps[:, 0:1536])
                            half = slice(hp * 1536, (hp + 1) * 1536)
                            nc.vector.tensor_add(
                                out=uhe[:], in0=uhe[:], in1=hsg[:, half]
                            )
                            z = sp.tile([128, 1536], BF16, tag="z", bufs=2)
                            nc.scalar.activation(
                                out=z[:], in_=uhe[:], func=AF.Prelu, alpha=NEG_SLOPE
                            )
                            za = sp.tile([128, 1536], BF16, tag="za", bufs=1)
                            nc.vector.tensor_mul(
                                out=za[:],
                                in0=z[:],
                                in1=att_t[:, hp * 1536 : (hp + 1) * 1536],
                            )
                            for hh in range(2):
                                h = hp * 2 + hh
                                nc.vector.tensor_reduce(
                                    out=logit[:, c * H + h : c * H + h + 1],
                                    in_=za[:, hh * D : (hh + 1) * D],
                                    axis=mybir.AxisListType.X,
                                    op=ALU.add,
                                )

                    # p = exp(logits), bf16 copy for den rhs
                    p_t = sp.tile([128, H * maxc], F32, tag="pt", bufs=2)
                    pb_t = sp.tile([128, H * maxc], BF16, tag="ptb", bufs=2)
                    nc.scalar.activation(
                        out=p_t[:, 0 : H * nch_w],
                        in_=logit[:, 0 : H * nch_w],
                        func=AF.Exp,
                    )
                    nc.vector.tensor_copy(
                        out=pb_t[:, 0 : H * nch_w], in_=p_t[:, 0 : H * nch_w]
                    )
                    psb = sp.tile([128, H], BF16, tag="psb", bufs=2)
                    nc.vector.tensor_copy(
                        out=psb[:], in_=pself_all[:, 4 * w : 4 * w + 4]
                    )

                    # ---- scatter + finalize (per head-pair pass) ----
                    acc = sp.tile([128, D], F32, tag="acc", bufs=2)
                    for hp in range(2):
                        po = pp.tile([128, 2048], F32, tag="psA", bufs=2)
                        nchunks = nch_w + 1  # + self-loop diag
                        for c in range(nchunks):
                            last = c == nchunks - 1
                            if last:
                                # self-loop: v = pself_h * hs_win, lhsT = I
                                v = sp.tile([128, 1536], BF16, tag="v", bufs=2)
                                for hh in range(2):
                                    h = hp * 2 + hh
                                    nc.vector.tensor_scalar_mul(
                                        v[:, hh * D : (hh + 1) * D],
                                        in0=hsw2[:, h * D : (h + 1) * D],
                                        scalar1=pself_all[:, 4 * w + h : 4 * w + h + 1],
                                    )
                                lhsT = identb[:]
                                pcols = psb[:, hp * 2 : hp * 2 + 2]
                            else:
                                v = sp.tile([128, 1536], BF16, tag="v", bufs=2)
                                for hh in range(2):
                                    h = hp * 2 + hh
                                    nc.vector.tensor_scalar_mul(
                                        v[:, hh * D : (hh + 1) * D],
                                        in0=hsgs[c][:, (hp * 2 + hh) * D : (hp * 2 + hh + 1) * D],
                                        scalar1=p_t[:, c * H + h : c * H + h + 1],
                                    )
                                lhsT = eqs[c][:]
                                pcols = pb_t[:, c * H + hp * 2 : c * H + hp * 2 + 2]
                            for s in range(3):
                                nc.tensor.matmul(
                                    po[:, s * 512 : (s + 1) * 512],
                                    lhsT=lhsT,
                                    rhs=v[:, s * 512 : (s + 1) * 512],
                                    start=(c == 0),
                                    stop=last,
                                )
                            nc.tensor.matmul(
                                po[:, 1536:1538],
                                lhsT=lhsT,
                                rhs=pcols,
                                start=(c == 0),
                                stop=last,
                            )
                        rden = sp.tile([128, 2], F32, tag="rden", bufs=2)
                        nc.vector.reciprocal(out=rden[:], in_=po[:, 1536:1538])
                        nc.vector.tensor_scalar_mul(
                            rden[:], in0=rden[:], scalar1=1.0 / H
                        )
                        if hp == 0:
                            nc.scalar.activation(
                                out=acc[:],
                                in_=po[:, 0:D],
                                func=AF.Copy,
                                scale=rden[:, 0:1],
                            )
                        else:
                            nc.vector.scalar_tensor_tensor(
                                out=acc[:],
                                in0=po[:, 0:D],
                                scalar=rden[:, 0:1],
                                in1=acc[:],
                                op0=ALU.mult,
                                op1=ALU.add,
                            )
                        nc.vector.scalar_tensor_tensor(
                            out=acc[:],
                            in0=po[:, D : 2 * D],
                            scalar=rden[:, 1:2],
                            in1=acc[:],
                            op0=ALU.mult,
                            op1=ALU.add,
                        )

                    # ---- final per window ----
                    nc.vector.tensor_add(out=acc[:], in0=acc[:], in1=bias_b[:])
                    g_t = sp.tile([128, D], F32, tag="fin", bufs=2)
                    nc.scalar.activation(out=g_t[:], in_=acc[:], func=AF.Gelu_apprx_tanh)
                    nc.vector.tensor_add(out=g_t[:], in0=g_t[:], in1=xc[:])
                    nc.sync.dma_start(out=xb[l % 2][rows, :], in_=g_t[:])
                    # xT(l+1) blocks for this window via PE transpose
                    pt = pp.tile([128, 2048], F32, tag="psA", bufs=2)
                    for kt in range(6):
                        nc.tensor.transpose(
                            out=pt[:, kt * 128 : (kt + 1) * 128],
                            in_=g_t[:, kt * 128 : (kt + 1) * 128],
                            identity=ident[:],
                        )
                    nc.scalar.copy(
                        out=xT_nxt[:]
                        .rearrange("p (k n) -> p k n", k=6)[:, :, w * 128 : (w + 1) * 128],
                        in_=pt[:, 0:768].rearrange("p (k n) -> p k n", n=128),
                    )
                    if l < L - 1:
                        # hs(l+1) for this window -> ag_in
                        for fh in range(2):
                            psh = pp.tile([128, 2048], F32, tag="psA", bufs=2)
                            for kt in range(6):
                                for s in range(3):
                                    nc.tensor.matmul(
                                        psh[:, s * 512 : (s + 1) * 512],
                                        lhsT=xT_nxt[
                                            :, kt * NL + w * 128 : kt * NL + (w + 1) * 128
                                        ],
                                        rhs=wsn[fh][
                                            :, kt * 1536 + s * 512 : kt * 1536 + (s + 1) * 512
                                        ],
                                        start=(kt == 0),
                                        stop=(kt == 5),
                                    )
                            evh = sp.tile([128, 1536], BF16, tag="stage", bufs=2)
                            nc.scalar.copy(out=evh[:], in_=psh[:, 0:1536])
                            nc.sync.dma_start(
                                out=ag_in[rows, fh * 1536 : (fh + 1) * 1536],
                                in_=evh[:],
                            )
                    ci += nch_w

                x_cur = xb[l % 2]
                xT = xT_nxt

            # ================= projection + LayerNorm + gelu =================
            pwt = sp.tile([128, 6 * D], BF16, tag="wt", bufs=2)
            for kt in range(6):
                nc.sync.dma_start(
                    out=pwt[:, kt * D : (kt + 1) * D],
                    in_=pw_in[kt * 128 : (kt + 1) * 128, :],
                )
            for m in range(NW):
                rows = slice(m * 128, (m + 1) * 128)
                ps = pp.tile([128, 2048], F32, tag="psA", bufs=2)
                for kt in range(6):
                    for a, b in ((0, 512), (512, 768)):
                        nc.tensor.matmul(
                            ps[:, a:b],
                            lhsT=xT[:, kt * NL + m * 128 : kt * NL + (m + 1) * 128],
                            rhs=pwt[:, kt * D + a : kt * D + b],
                            start=(kt == 0),
                            stop=(kt == 5),
                        )
                y0 = sp.tile([128, D], F32, tag="acc", bufs=2)
                nc.vector.tensor_add(out=y0[:], in0=ps[:, 0:D], in1=pbb[:])
                mu = sp.tile([128, 1], F32, tag="stats", bufs=4)
                nc.vector.tensor_reduce(
                    out=mu[:], in_=y0[:], axis=mybir.AxisListType.X, op=ALU.add
                )
                nc.vector.tensor_scalar_mul(mu[:], in0=mu[:], scalar1=1.0 / D)
                xc2 = sp.tile([128, D], F32, tag="fin", bufs=2)
                nc.vector.tensor_scalar_sub(xc2[:], in0=y0[:], scalar1=mu[:])
                var = sp.tile([128, 1], F32, tag="stats", bufs=4)
                nc.vector.scalar_tensor_tensor(
                    out=y0[:],
                    in0=xc2[:],
                    scalar=1.0,
                    in1=xc2[:],
                    op0=ALU.mult,
                    op1=ALU.mult,
                    accum_out=var[:],
                )
                nc.vector.tensor_scalar(
                    var[:], in0=var[:], scalar1=1.0 / D, scalar2=LN_EPS,
                    op0=ALU.mult, op1=ALU.add,
                )
                sd = sp.tile([128, 1], F32, tag="stats", bufs=4)
                nc.scalar.activation(out=sd[:], in_=var[:], func=AF.Sqrt)
                rstd = sp.tile([128, 1], F32, tag="stats", bufs=4)
                nc.vector.reciprocal(out=rstd[:], in_=sd[:])
                nc.vector.tensor_scalar_mul(y0[:], in0=xc2[:], scalar1=rstd[:])
                nc.vector.tensor_mul(out=y0[:], in0=y0[:], in1=lngb[:])
                nc.vector.tensor_add(out=y0[:], in0=y0[:], in1=lnbb[:])
                og = sp.tile([128, D], F32, tag="fin", bufs=2)
                nc.scalar.activation(out=og[:], in_=y0[:], func=AF.Gelu_apprx_tanh)
                nc.sync.dma_start(out=out_t[rows, :], in_=og[:])

    _split_multi_waits(nc)
    return nc


# ----------------------------------------------------------------------------
# Host side
# ----------------------------------------------------------------------------


def _preprocess(edge_index, edge_attr):
    src = np.asarray(edge_index[0], dtype=np.int64)
    dst = np.asarray(edge_index[1], dtype=np.int64)
    attr = np.asarray(edge_attr, dtype=np.int64)

    deg = np.bincount(dst, minlength=N).astype(np.float32)
    C = np.zeros((N, R), np.float32)
    np.add.at(C, (dst, attr), 1.0)
    Cdiv = C / np.maximum(deg, 1.0)[:, None]

    win = dst // WSZ  # global window id, 0..127
    order = np.argsort(win, kind="stable")
    src_s, dst_s, attr_s = src[order], dst[order], attr[order]
    wcnt = np.bincount(win, minlength=N // WSZ)
    wstart = np.concatenate([[0], np.cumsum(wcnt)])

    cpw = []
    for w in range(NW):
        mx = 1
        for k in range(NCORES):
            gw = k * NW + w
            mx = max(mx, -(-int(wcnt[gw]) // 128))
        cpw.append(int(mx))
    nch = sum(cpw)

    idx_all, rep_all = [], []
    for k in range(NCORES):
        arr = np.zeros((nch, 128, 2), np.int32)
        arr[:, :, 1] = 255  # padding dstlocal: one-hot never matches
        rep = np.zeros((nch, 256), np.int32)
        rep[:, 0:128] = 255
        ptr = 0
        for w in range(NW):
            gw = k * NW + w
            base = gw * WSZ
            s0, s1 = int(wstart[gw]), int(wstart[gw + 1])
            es, ed, ea = src_s[s0:s1], dst_s[s0:s1], attr_s[s0:s1]
            nreal = s1 - s0
            rows = np.zeros((cpw[w] * 128, 2), np.int32)
            rows[:, 1] = 255
            rows[:nreal, 0] = es
            rows[:nreal, 1] = ed - base
            rrows = np.zeros((cpw[w], 128, 2), np.int32)
            rrows[:, :, :] = rows.reshape(cpw[w], 128, 2)
            arr[ptr : ptr + cpw[w]] = rrows
            rr = np.zeros((cpw[w] * 128, 2), np.int32)
            rr[:, 0] = 255
            rr[:nreal, 0] = ed - base
            rr[:nreal, 1] = ea
            rep[ptr : ptr + cpw[w], 0:128] = rr[:, 0].reshape(cpw[w], 128)
            rep[ptr : ptr + cpw[w], 128:256] = rr[:, 1].reshape(cpw[w], 128)
            ptr += cpw[w]
        idx_all.append(
            np.ascontiguousarray(arr.transpose(1, 0, 2).reshape(128, nch * 2))
        )
        repb = rep.reshape(1, nch * 256).astype(ml_dtypes.bfloat16)
        rep_all.append(np.ascontiguousarray(np.broadcast_to(repb, (128, nch * 256))))
    return cpw, nch, idx_all, rep_all, Cdiv


_cache = {}
LAST_RESULTS = None
LAST_EXEC_NS = None
LAST_RES = None


def kernel(**inputs):
    x = np.asarray(inputs["x"], np.float32)
    rel_emb = np.asarray(inputs["rel_emb"], np.float32)
    w_src = np.asarray(inputs["w_src"], np.float32)
    w_dst = np.asarray(inputs["w_dst"], np.float32)
    w_edge = np.asarray(inputs["w_edge"], np.float32)
    att = np.asarray(inputs["att"], np.float32)
    bias = np.asarray(inputs["bias"], np.float32)
    proj_w = np.asarray(inputs["proj_w"], np.float32)
    proj_b = np.asarray(inputs["proj_b"], np.float32)
    ln_g = np.asarray(inputs["ln_g"], np.float32)
    ln_b = np.asarray(inputs["ln_b"], np.float32)
    edge_index = np.asarray(inputs["edge_index"], np.int32)
    edge_attr = np.asarray(inputs["edge_attr"], np.int32)

    cpw, nch, idx_all, rep_all, Cdiv = _preprocess(edge_index, edge_attr)

    key = (tuple(cpw), nch)
    if key not in _cache:
        _cache[key] = build_program(cpw, nch)
    nc = _cache[key]

    bf = lambda a: np.ascontiguousarray(a).astype(ml_dtypes.bfloat16)
    ws_b = bf(w_src.reshape(L, D, HD))
    wd_b = bf(w_dst.reshape(L, D, HD))
    we_b = bf(w_edge.reshape(L, D, HD))
    att_rep = bf(np.broadcast_to(att.reshape(L, 1, HD), (L, 128, HD)))
    bias_rep = np.ascontiguousarray(
        np.broadcast_to(bias.reshape(L, 1, D), (L, 128, D)), dtype=np.float32
    )
    pw_b = bf(proj_w)
    pb_rep = np.ascontiguousarray(np.broadcast_to(proj_b, (128, D)), np.float32)
    lng_rep = np.ascontiguousarray(np.broadcast_to(ln_g, (128, D)), np.float32)
    lnb_rep = np.ascontiguousarray(np.broadcast_to(ln_b, (128, D)), np.float32)

    in_maps = []
    for k in range(NCORES):
        rows = slice(k * NL, (k + 1) * NL)
        xs = np.ascontiguousarray(x[rows])
        xsT = xs.T.astype(ml_dtypes.bfloat16)  # [768, 2048]
        xT0 = np.ascontiguousarray(
            np.concatenate([xsT[kt * 128 : (kt + 1) * 128, :] for kt in range(6)], axis=1)
        )
        in_maps.append(
            {
                "x_own": xs,
                "xT0": xT0,
                "idx": idx_all[k],
                "rep2": rep_all[k],
                "cdivT": bf(Cdiv[rows].T),
                "w_src": ws_b,
                "w_dst": wd_b,
                "w_edge": we_b,
                "rel_emb": np.ascontiguousarray(rel_emb, np.float32),
                "att_rep": att_rep,
                "bias_rep": bias_rep,
                "proj_w": pw_b,
                "pb_rep": bf(pb_rep),
                "lng_rep": bf(lng_rep),
                "lnb_rep": bf(lnb_rep),
            }
        )

    trace = os.environ.get("GAT_TRACE", "0") == "1"
    res = run_bass_kernel_spmd(
        nc, in_maps, core_ids=list(range(NCORES)), trace=trace
    )
    global LAST_RESULTS, LAST_EXEC_NS, LAST_RES
    LAST_RESULTS = res.results
    LAST_EXEC_NS = res.exec_time_ns
    LAST_RES = res
    out = np.concatenate([res.results[k]["out"] for k in range(NCORES)], axis=0)
    return out.astype(np.float32)


# revision 14
# speedup vs baseline: 1.0966x; 1.0966x over previous
"""Trainium2 Bass kernel for nn_ExpaModel_73478300500036 (3-layer GATv2-style
GNN message passing, N=16384 nodes, E=49152 edges, D=768, H=4 heads).

Strategy (8 NeuronCores, SPMD, dst-sharded):
  - core k owns nodes [k*2048, (k+1)*2048) and computes their output rows.
  - Per layer: hs = x @ Wsrc computed for the own shard, AllGathered to all
    cores (the only collective); hd, he stay local.
  - Real edges grouped by 128-node dst windows, 128 edges/chunk.  Per chunk:
    one indirect row-gather of hs[src]; hd[dst] and he[attr] come from
    one-hot permutation matmuls on the PE (no DMA gathers):
      u = hsg + eqT_dst.T @ hd_win + eqT_attr.T @ he_table
    z = leaky(u), logits via per-head multiply+reduce, p = exp(logits)
    (segment max skipped: logits are O(10), exp is safe in fp32 and
    softmax is shift-invariant).
  - Scatter via one-hot matmuls: po += eq.T @ (p_h * hsg), den via rhs=p.
  - Self-loops (attr = mean of incoming) handled densely per window:
    z_self = leaky(hs_win + hd_win + Cdiv @ he_table); their scatter
    contribution enters the same PSUM groups via lhsT=identity.
  - PSUM: a single rotating pair of [128,2048] fp32 slots (4 banks each)
    serves weight matmuls, permute accumulation, and scatter+den.
  - Projection + LayerNorm + gelu data-parallel over nodes.

Storage dtype bf16, fp32 accumulation.
"""

import os
import sys

sys.path.insert(0, "/opt/trn_rl_repo")

# The RDH collective algorithm (1-56MB messages) crashes the device in this
# environment; force mesh/ring instead.
os.environ.setdefault("NEURON_RT_DBG_RDH_CC", "0")

import ml_dtypes
import numpy as np

import concourse.bass as bass
import concourse.mybir as mybir
import concourse.tile as tile
from concourse.bass_utils import run_bass_kernel_spmd
from concourse.masks import make_identity
from concourse.vector_clock import ScopedClock

# ----------------------------------------------------------------------------
# Workaround: this container's walrus build supports at most ONE sync wait per
# instruction. (a) Tile's tail drain carries several waits -> emit them as
# separate SP EventSemaphore waits; (b) post-pass splits any remaining
# multi-wait instruction.
# ----------------------------------------------------------------------------


def _patched_drain_and_barrier(self, tick_clock, wait_clock):
    nc = self.nc
    probe = mybir.InstDrain(
        name=nc.get_next_instruction_name(), ins=[], outs=[], bass_is_fusable=False
    )
    probe.engine = mybir.EngineType.SP
    wait_clock.add_sem_waits(probe, ScopedClock({None: tick_clock.global_clock}))
    waits = []
    si = probe.sync_info
    if si is not None and si.on_wait:
        waits = list(si.on_wait)
    id2sem = {h.num: h for h in self.sems.allocated().values()}
    for w in waits:
        sem = id2sem.get(w.id)
        assert sem is not None, f"drain wait on unknown sem id {w.id}"
        nc.sync.wait_ge(sem, w.wait_value)
    nc.sync.drain()
    nc.all_engine_barrier()
    assert self.sems is not None
    popped = nc._tile_sem_poison_stack.pop()
    assert popped is self._sem_poison
    nc.clear_and_free_semaphores(list(self.sems.allocated().values()))
    nc.all_engine_barrier()


tile.TileContext._drain_and_barrier = _patched_drain_and_barrier

_split_n = [0]


def _split_multi_waits(nc):
    for f in nc.m.functions:
        for bb in f.blocks:
            insts = list(bb.instructions)
            changed = False
            new_list = []
            for inst in insts:
                si = inst.sync_info
                waits = list(si.on_wait) if (si is not None and si.on_wait) else []
                if len(waits) > 1:
                    changed = True
                    for w in waits[:-1]:
                        _split_n[0] += 1
                        ev = mybir.InstEventSemaphore(
                            name=f"evsplit-{_split_n[0]}", ins=[], outs=[]
                        )
                        ev.engine = inst.engine
                        ev.sync_info = mybir.SyncInfo(on_wait=[w], on_update=[])
                        new_list.append(ev)
                    inst.sync_info = mybir.SyncInfo(
                        on_wait=[waits[-1]],
                        on_update=list(si.on_update) if si.on_update else [],
                    )
                new_list.append(inst)
            if changed:
                bb.instructions = new_list


# ----------------------------------------------------------------------------
# Problem constants (hardcoded per spec)
# ----------------------------------------------------------------------------
NCORES = 8
N = 16384
E = 49152
D = 768
H = 4
L = 3
R = 64
HD = H * D  # 3072
HH = HD // 2  # 1536 (head-pair half)
NL = N // NCORES  # 2048
NW = NL // 128  # 16 windows per core
WSZ = 128
LN_EPS = 1e-5
NEG_SLOPE = 0.2

F32 = mybir.dt.float32
BF16 = mybir.dt.bfloat16
I32 = mybir.dt.int32

AF = mybir.ActivationFunctionType
ALU = mybir.AluOpType


def build_program(cpw, nch):
    nc = bass.Bass("TRN2", num_devices=NCORES)
    maxc = max(cpw)

    # ---- I/O ----
    x_in = nc.dram_tensor("x_own", [NL, D], F32, kind="ExternalInput")
    xT0_in = nc.dram_tensor("xT0", [128, 6 * NL], BF16, kind="ExternalInput")
    idx_in = nc.dram_tensor("idx", [128, nch * 2], I32, kind="ExternalInput")
    rep_in = nc.dram_tensor("rep2", [128, nch * 256], BF16, kind="ExternalInput")
    cdt_in = nc.dram_tensor("cdivT", [R, NL], BF16, kind="ExternalInput")
    ws_in = nc.dram_tensor("w_src", [L, D, HD], BF16, kind="ExternalInput")
    wd_in = nc.dram_tensor("w_dst", [L, D, HD], BF16, kind="ExternalInput")
    we_in = nc.dram_tensor("w_edge", [L, D, HD], BF16, kind="ExternalInput")
    rel_in = nc.dram_tensor("rel_emb", [R, D], F32, kind="ExternalInput")
    att_in = nc.dram_tensor("att_rep", [L, 128, HD], BF16, kind="ExternalInput")
    bias_in = nc.dram_tensor("bias_rep", [L, 128, D], F32, kind="ExternalInput")
    pw_in = nc.dram_tensor("proj_w", [D, D], BF16, kind="ExternalInput")
    pb_in = nc.dram_tensor("pb_rep", [128, D], BF16, kind="ExternalInput")
    lng_in = nc.dram_tensor("lng_rep", [128, D], BF16, kind="ExternalInput")
    lnb_in = nc.dram_tensor("lnb_rep", [128, D], BF16, kind="ExternalInput")
    out_t = nc.dram_tensor("out", [NL, D], F32, kind="ExternalOutput")

    # ---- internal DRAM ----
    ag_in = nc.dram_tensor("ag_in", [NL, HD], BF16, kind="Internal")
    hs_full = nc.dram_tensor(
        "hs_full", [N, HD], BF16, kind="Internal", addr_space="Shared"
    )
    hd_dram = nc.dram_tensor("hd_dram", [NL, HD], BF16, kind="Internal")
    xb = [nc.dram_tensor(f"xb{i}", [NL, D], F32, kind="Internal") for i in range(2)]
    xbf = [nc.dram_tensor(f"xbf{i}", [NL, D], BF16, kind="Internal") for i in range(2)]

    with tile.TileContext(nc) as tc:
        with (
            tc.tile_pool(name="sb", bufs=1) as sp,
            tc.tile_pool(name="ps", bufs=1, space="PSUM") as pp,
        ):
            # ---- static tiles ----
            ident = sp.tile([128, 128], F32, tag="ident")
            make_identity(nc, ident[:])
            identb = sp.tile([128, 128], BF16, tag="identb")
            nc.vector.tensor_copy(out=identb[:], in_=ident[:])
            ioti = sp.tile([128, 128], mybir.dt.int16, tag="ioti")
            nc.gpsimd.iota(ioti[:], pattern=[[1, 128]], base=0, channel_multiplier=0)
            iotab = sp.tile([128, 128], BF16, tag="iotab")
            nc.vector.tensor_copy(out=iotab[:], in_=ioti[:])
            ioPi = sp.tile([128, 1], I32, tag="ioPi")
            nc.gpsimd.iota(ioPi[:], pattern=[[0, 1]], base=0, channel_multiplier=1)
            ioPb = sp.tile([128, 1], BF16, tag="ioPb")
            nc.vector.tensor_copy(out=ioPb[:], in_=ioPi[:])
            ones_b = sp.tile([128, 1], BF16, tag="ones")
            nc.vector.memset(ones_b[:], 1.0)

            idx_t = sp.tile([128, nch * 2], I32, tag="idx")
            nc.sync.dma_start(out=idx_t[:], in_=idx_in[:])
            idx2 = idx_t[:].rearrange("p (c f) -> p c f", f=2)

            cdt = sp.tile([R, NL], BF16, tag="cdt")
            nc.sync.dma_start(out=cdt[:], in_=cdt_in[:])
            rel_sb = sp.tile([R, D], F32, tag="rel")
            nc.sync.dma_start(out=rel_sb[:], in_=rel_in[:])

            # relT [768, 64] as 6 blocks [128, 64], via PE transpose (fp32)
            relT = sp.tile([128, 6 * R], BF16, tag="relT")
            for kt in range(6):
                pt = pp.tile([128, 2048], F32, tag="psA", bufs=2)
                nc.tensor.transpose(
                    out=pt[:, 0:R],
                    in_=rel_sb[:, kt * 128 : (kt + 1) * 128],
                    identity=ident[:R, :R],
                )
                nc.scalar.copy(out=relT[:, kt * R : (kt + 1) * R], in_=pt[:, 0:R])

            pbb = sp.tile([128, D], BF16, tag="pbb")
            nc.sync.dma_start(out=pbb[:], in_=pb_in[:])
            lngb = sp.tile([128, D], BF16, tag="lngb")
            nc.sync.dma_start(out=lngb[:], in_=lng_in[:])
            lnbb = sp.tile([128, D], BF16, tag="lnbb")
            nc.sync.dma_start(out=lnbb[:], in_=lnb_in[:])

            het = sp.tile([R, HD], BF16, tag="het")
            pself_all = sp.tile([128, 4 * NW], F32, tag="pself")

            def wload(w_dram, l, fh):
                wt = sp.tile([128, 6 * 1536], BF16, tag="wt", bufs=2)
                for kt in range(6):
                    nc.sync.dma_start(
                        out=wt[:, kt * 1536 : (kt + 1) * 1536],
                        in_=w_dram[
                            l, kt * 128 : (kt + 1) * 128, fh * 1536 : (fh + 1) * 1536
                        ],
                    )
                return wt

            def proj_half(xT, wt, fh, dest):
                """dest[:, fh half] <- x @ W (one 1536-col half)."""
                for m in range(NW):
                    ps = pp.tile([128, 2048], F32, tag="psA", bufs=2)
                    for kt in range(6):
                        for s in range(3):
                            nc.tensor.matmul(
                                ps[:, s * 512 : (s + 1) * 512],
                                lhsT=xT[:, kt * NL + m * 128 : kt * NL + (m + 1) * 128],
                                rhs=wt[:, kt * 1536 + s * 512 : kt * 1536 + (s + 1) * 512],
                                start=(kt == 0),
                                stop=(kt == 5),
                            )
                    ev = sp.tile([128, 1536], BF16, tag="stage", bufs=2)
                    nc.scalar.copy(out=ev[:], in_=ps[:, 0:1536])
                    nc.sync.dma_start(
                        out=dest[m * 128 : (m + 1) * 128, fh * 1536 : (fh + 1) * 1536],
                        in_=ev[:],
                    )

            x_cur = x_in
            xbf_prev = None
            for l in range(L):
                # ---- xT ----
                xT = sp.tile([128, 6 * NL], BF16, tag="xT")
                if l == 0:
                    nc.sync.dma_start(out=xT[:], in_=xT0_in[:])
                else:
                    for kt in range(6):
                        nc.sync.dma_start_transpose(
                            out=xT[:, kt * NL : (kt + 1) * NL],
                            in_=xbf_prev[:, kt * 128 : (kt + 1) * 128],
                        )

                att_t = sp.tile([128, HD], BF16, tag="att", bufs=1)
                nc.sync.dma_start(out=att_t[:], in_=att_in[l])
                bias_b = sp.tile([128, D], F32, tag="biasb", bufs=1)
                nc.sync.dma_start(out=bias_b[:], in_=bias_in[l])

                # ---- hs -> ag_in, then AllGather ----
                for fh in range(2):
                    wt = wload(ws_in, l, fh)
                    proj_half(xT, wt, fh, ag_in)
                # ---- hd -> hd_dram (overlaps AllGather) ----
                for fh in range(2):
                    wt = wload(wd_in, l, fh)
                    proj_half(xT, wt, fh, hd_dram)

                # ---- he table: het = rel_emb @ Wedge  [64, 3072] ----
                for fh in range(2):
                    wt = wload(we_in, l, fh)
                    ps = pp.tile([128, 2048], F32, tag="psA", bufs=2)
                    for kt in range(6):
                        for s in range(3):
                            nc.tensor.matmul(
                                ps[:R, s * 512 : (s + 1) * 512],
                                lhsT=relT[:, kt * R : (kt + 1) * R],
                                rhs=wt[:, kt * 1536 + s * 512 : kt * 1536 + (s + 1) * 512],
                                start=(kt == 0),
                                stop=(kt == 5),
                            )
                    nc.scalar.copy(
                        out=het[:, fh * 1536 : (fh + 1) * 1536], in_=ps[:R, 0:1536]
                    )

                # ---- self-loop pass per window (overlaps AllGather) ----
                for w in range(NW):
                    rows = slice(w * 128, (w + 1) * 128)
                    slog = sp.tile([128, H], F32, tag="slog", bufs=2)
                    for hp in range(2):
                        csl = slice(hp * 1536, (hp + 1) * 1536)
                        hsw = sp.tile([128, 1536], BF16, tag="hswA", bufs=2)
                        nc.sync.dma_start(out=hsw[:], in_=ag_in[rows, csl])
                        hdw = sp.tile([128, 1536], BF16, tag="hdwA", bufs=2)
                        nc.sync.dma_start(out=hdw[:], in_=hd_dram[rows, csl])
                        ps = pp.tile([128, 2048], F32, tag="psA", bufs=2)
                        for s in range(3):
                            nc.tensor.matmul(
                                ps[:, s * 512 : (s + 1) * 512],
                                lhsT=cdt[:, w * 128 : (w + 1) * 128],
                                rhs=het[:, hp * 1536 + s * 512 : hp * 1536 + (s + 1) * 512],
                                start=True,
                                stop=True,
                            )
                        hl = sp.tile([128, 1536], BF16, tag="uhe", bufs=2)
                        nc.scalar.copy(out=hl[:], in_=ps[:, 0:1536])
                        nc.vector.tensor_add(out=hl[:], in0=hl[:], in1=hsw[:])
                        nc.vector.tensor_add(out=hl[:], in0=hl[:], in1=hdw[:])
                        z = sp.tile([128, 1536], BF16, tag="z", bufs=2)
                        nc.scalar.activation(
                            out=z[:], in_=hl[:], func=AF.Prelu, alpha=NEG_SLOPE
                        )
                        sc = sp.tile([128, D], BF16, tag="za", bufs=1)
                        for hh in range(2):
                            h = hp * 2 + hh
                            nc.vector.scalar_tensor_tensor(
                                out=sc[:],
                                in0=z[:, hh * D : (hh + 1) * D],
                                scalar=1.0,
                                in1=att_t[:, h * D : (h + 1) * D],
                                op0=ALU.mult,
                                op1=ALU.mult,
                                accum_out=slog[:, h : h + 1],
                            )
                    nc.scalar.activation(
                        out=pself_all[:, 4 * w : 4 * w + 4], in_=slog[:], func=AF.Exp
                    )

                nc.gpsimd.collective_compute(
                    "AllGather",
                    ALU.bypass,
                    ins=[ag_in[:]],
                    outs=[hs_full[:]],
                    replica_groups=[list(range(NCORES))],
                )

                # ---- edge phase ----
                ci = 0
                for w in range(NW):
                    rows = slice(w * 128, (w + 1) * 128)
                    nch_w = cpw[w]
                    hsw2 = sp.tile([128, HD], BF16, tag="hswB", bufs=2)
                    nc.sync.dma_start(out=hsw2[:], in_=ag_in[rows, :])
                    hdw2 = sp.tile([128, HD], BF16, tag="hdwB", bufs=2)
                    nc.sync.dma_start(out=hdw2[:], in_=hd_dram[rows, :])
                    xc = sp.tile([128, D], F32, tag="xc", bufs=1)
                    nc.sync.dma_start(out=xc[:], in_=x_cur[rows, :])
                    rep_t = sp.tile([128, maxc * 256], BF16, tag="rep", bufs=1)
                    nc.sync.dma_start(
                        out=rep_t[:, 0 : nch_w * 256],
                        in_=rep_in[:, ci * 256 : (ci + nch_w) * 256],
                    )

                    hsgs = []
                    eqs = []
                    ps_list = []
                    logit = sp.tile([128, H * maxc], F32, tag="logit", bufs=2)
                    for c in range(nch_w):
                        cidx = ci + c
                        hsg = sp.tile([128, HD], BF16, tag="hsg", bufs=maxc + 1)
                        nc.gpsimd.indirect_dma_start(
                            out=hsg[:],
                            out_offset=None,
                            in_=hs_full[:, :],
                            in_offset=bass.IndirectOffsetOnAxis(
                                ap=idx2[:, cidx, 0:1], axis=0
                            ),
                        )
                        hsgs.append(hsg)
                        dstf = sp.tile([128, 1], BF16, tag="dstf", bufs=3)
                        nc.vector.tensor_copy(out=dstf[:], in_=idx2[:, cidx, 1:2])
                        eq = sp.tile([128, 128], BF16, tag="eq", bufs=maxc + 2)
                        nc.vector.tensor_tensor(
                            out=eq[:],
                            in0=dstf[:].to_broadcast([128, 128]),
                            in1=iotab[:],
                            op=ALU.is_equal,
                        )
                        eqs.append(eq)
                        eqT = sp.tile([128, 128], BF16, tag="eqT", bufs=3)
                        nc.vector.tensor_tensor(
                            out=eqT[:],
                            in0=ioPb[:].to_broadcast([128, 128]),
                            in1=rep_t[:, c * 256 : c * 256 + 128],
                            op=ALU.is_equal,
                        )
                        eqh = sp.tile([128, 128], BF16, tag="eqh", bufs=3)
                        nc.vector.tensor_tensor(
                            out=eqh[:R, :],
                            in0=ioPb[:R, :].to_broadcast([R, 128]),
                            in1=rep_t[:R, c * 256 + 128 : c * 256 + 256],
                            op=ALU.is_equal,
                        )
                        for hp in range(2):
                            ps = pp.tile([128, 2048], F32, tag="psA", bufs=2)
                            for s in range(3):
                                nc.tensor.matmul(
                                    ps[:, s * 512 : (s + 1) * 512],
                                    lhsT=eqT[:],
                                    rhs=hdw2[
                                        :, hp * 1536 + s * 512 : hp * 1536 + (s + 1) * 512
                                    ],
                                    start=True,
                                    stop=False,
                                )
                            for s in range(3):
                                nc.tensor.matmul(
                                    ps[:, s * 512 : (s + 1) * 512],
                                    lhsT=eqh[:R, :],
                                    rhs=het[
                                        :, hp * 1536 + s * 512 : hp * 1536 + (s + 1) * 512
                                    ],
                                    start=False,
                                    stop=True,
                                )
                            uhe = sp.tile([128, 1536], BF16, tag="uhe", bufs=2)
                            nc.scalar.copy(out=uhe[:], in_=ps[:, 0:1536])
                            half = slice(hp * 1536, (hp + 1) * 1536)
                            nc.vector.tensor_add(
                                out=uhe[:], in0=uhe[:], in1=hsg[:, half]
                            )
                            z = sp.tile([128, 1536], BF16, tag="z", bufs=2)
                            nc.scalar.activation(
                                out=z[:], in_=uhe[:], func=AF.Prelu, alpha=NEG_SLOPE
                            )
                            sc = sp.tile([128, D], BF16, tag="za", bufs=1)
                            for hh in range(2):
                                h = hp * 2 + hh
                                nc.vector.scalar_tensor_tensor(
                                    out=sc[:],
                                    in0=z[:, hh * D : (hh + 1) * D],
                                    scalar=1.0,
                                    in1=att_t[:, h * D : (h + 1) * D],
                                    op0=ALU.mult,
                                    op1=ALU.mult,
                                    accum_out=logit[:, c * H + h : c * H + h + 1],
                                )

                    # p = exp(logits), bf16 copy for den rhs
                    p_t = sp.tile([128, H * maxc], F32, tag="pt", bufs=2)
                    pb_t = sp.tile([128, H * maxc], BF16, tag="ptb", bufs=2)
                    nc.scalar.activation(
                        out=p_t[:, 0 : H * nch_w],
                        in_=logit[:, 0 : H * nch_w],
                        func=AF.Exp,
                    )
                    nc.vector.tensor_copy(
                        out=pb_t[:, 0 : H * nch_w], in_=p_t[:, 0 : H * nch_w]
                    )
                    psb = sp.tile([128, H], BF16, tag="psb", bufs=2)
                    nc.vector.tensor_copy(
                        out=psb[:], in_=pself_all[:, 4 * w : 4 * w + 4]
                    )

                    # ---- scatter + finalize (per head-pair pass) ----
                    acc = sp.tile([128, D], F32, tag="acc", bufs=2)
                    for hp in range(2):
                        po = pp.tile([128, 2048], F32, tag="psA", bufs=2)
                        nchunks = nch_w + 1  # + self-loop diag
                        for c in range(nchunks):
                            last = c == nchunks - 1
                            if last:
                                # self-loop: v = pself_h * hs_win, lhsT = I
                                v = sp.tile([128, 1536], BF16, tag="v", bufs=2)
                                for hh in range(2):
                                    h = hp * 2 + hh
                                    nc.vector.tensor_scalar_mul(
                                        v[:, hh * D : (hh + 1) * D],
                                        in0=hsw2[:, h * D : (h + 1) * D],
                                        scalar1=pself_all[:, 4 * w + h : 4 * w + h + 1],
                                    )
                                lhsT = identb[:]
                                pcols = psb[:, hp * 2 : hp * 2 + 2]
                            else:
                                v = sp.tile([128, 1536], BF16, tag="v", bufs=2)
                                for hh in range(2):
                                    h = hp * 2 + hh
                                    nc.vector.tensor_scalar_mul(
                                        v[:, hh * D : (hh + 1) * D],
                                        in0=hsgs[c][:, (hp * 2 + hh) * D : (hp * 2 + hh + 1) * D],
                                        scalar1=p_t[:, c * H + h : c * H + h + 1],
                                    )
                                lhsT = eqs[c][:]
                                pcols = pb_t[:, c * H + hp * 2 : c * H + hp * 2 + 2]
                            for s in range(3):
                                nc.tensor.matmul(
                                    po[:, s * 512 : (s + 1) * 512],
                                    lhsT=lhsT,
                                    rhs=v[:, s * 512 : (s + 1) * 512],
                                    start=(c == 0),
                                    stop=last,
                                )
                            nc.tensor.matmul(
                                po[:, 1536:1538],
                                lhsT=lhsT,
                                rhs=pcols,
                                start=(c == 0),
                                stop=last,
                            )
                        rden = sp.tile([128, 2], F32, tag="rden", bufs=2)
                        nc.vector.reciprocal(out=rden[:], in_=po[:, 1536:1538])
                        nc.vector.tensor_scalar_mul(
                            rden[:], in0=rden[:], scalar1=1.0 / H
                        )
                        if hp == 0:
                            nc.scalar.activation(
                                out=acc[:],
                                in_=po[:, 0:D],
                                func=AF.Copy,
                                scale=rden[:, 0:1],
                            )
                        else:
                            nc.vector.scalar_tensor_tensor(
                                out=acc[:],
                                in0=po[:, 0:D],
                                scalar=rden[:, 0:1],
                                in1=acc[:],
                                op0=ALU.mult,
                                op1=ALU.add,
                            )
                        nc.vector.scalar_tensor_tensor(
                            out=acc[:],
                            in0=po[:, D : 2 * D],
                            scalar=rden[:, 1:2],
                            in1=acc[:],
                            op0=ALU.mult,
                            op1=ALU.add,
                        )

                    # ---- final per window ----
                    nc.vector.tensor_add(out=acc[:], in0=acc[:], in1=bias_b[:])
                    g_t = sp.tile([128, D], F32, tag="fin", bufs=2)
                    nc.scalar.activation(out=g_t[:], in_=acc[:], func=AF.Gelu_apprx_tanh)
                    nc.vector.tensor_add(out=g_t[:], in0=g_t[:], in1=xc[:])
                    nc.sync.dma_start(out=xb[l % 2][rows, :], in_=g_t[:])
                    gb = sp.tile([128, D], BF16, tag="finb", bufs=2)
                    nc.vector.tensor_copy(out=gb[:], in_=g_t[:])
                    nc.sync.dma_start(out=xbf[l % 2][rows, :], in_=gb[:])
                    ci += nch_w

                x_cur = xb[l % 2]
                xbf_prev = xbf[l % 2]

            # ================= projection + LayerNorm + gelu =================
            xT = sp.tile([128, 6 * NL], BF16, tag="xT")
            for kt in range(6):
                nc.sync.dma_start_transpose(
                    out=xT[:, kt * NL : (kt + 1) * NL],
                    in_=xbf_prev[:, kt * 128 : (kt + 1) * 128],
                )
            pwt = sp.tile([128, 6 * D], BF16, tag="wt", bufs=2)
            for kt in range(6):
                nc.sync.dma_start(
                    out=pwt[:, kt * D : (kt + 1) * D],
                    in_=pw_in[kt * 128 : (kt + 1) * 128, :],
                )
            for m in range(NW):
                rows = slice(m * 128, (m + 1) * 128)
                ps = pp.tile([128, 2048], F32, tag="psA", bufs=2)
                for kt in range(6):
                    for a, b in ((0, 512), (512, 768)):
                        nc.tensor.matmul(
                            ps[:, a:b],
                            lhsT=xT[:, kt * NL + m * 128 : kt * NL + (m + 1) * 128],
                            rhs=pwt[:, kt * D + a : kt * D + b],
                            start=(kt == 0),
                            stop=(kt == 5),
                        )
                y0 = sp.tile([128, D], F32, tag="acc", bufs=2)
                nc.vector.tensor_add(out=y0[:], in0=ps[:, 0:D], in1=pbb[:])
                mu = sp.tile([128, 1], F32, tag="stats", bufs=4)
                nc.vector.tensor_reduce(
                    out=mu[:], in_=y0[:], axis=mybir.AxisListType.X, op=ALU.add
                )
                nc.vector.tensor_scalar_mul(mu[:], in0=mu[:], scalar1=1.0 / D)
                xc2 = sp.tile([128, D], F32, tag="fin", bufs=2)
                nc.vector.tensor_scalar_sub(xc2[:], in0=y0[:], scalar1=mu[:])
                var = sp.tile([128, 1], F32, tag="stats", bufs=4)
                nc.vector.scalar_tensor_tensor(
                    out=y0[:],
                    in0=xc2[:],
                    scalar=1.0,
                    in1=xc2[:],
                    op0=ALU.mult,
                    op1=ALU.mult,
                    accum_out=var[:],
                )
                nc.vector.tensor_scalar(
                    var[:], in0=var[:], scalar1=1.0 / D, scalar2=LN_EPS,
                    op0=ALU.mult, op1=ALU.add,
                )
                sd = sp.tile([128, 1], F32, tag="stats", bufs=4)
                nc.scalar.activation(out=sd[:], in_=var[:], func=AF.Sqrt)
                rstd = sp.tile([128, 1], F32, tag="stats", bufs=4)
                nc.vector.reciprocal(out=rstd[:], in_=sd[:])
                nc.vector.tensor_scalar_mul(y0[:], in0=xc2[:], scalar1=rstd[:])
                nc.vector.tensor_mul(out=y0[:], in0=y0[:], in1=lngb[:])
                nc.vector.tensor_add(out=y0[:], in0=y0[:], in1=lnbb[:])
                og = sp.tile([128, D], F32, tag="fin", bufs=2)
                nc.scalar.activation(out=og[:], in_=y0[:], func=AF.Gelu_apprx_tanh)
                nc.sync.dma_start(out=out_t[rows, :], in_=og[:])

    _split_multi_waits(nc)
    return nc


# ----------------------------------------------------------------------------
# Host side
# ----------------------------------------------------------------------------


def _preprocess(edge_index, edge_attr):
    src = np.asarray(edge_index[0], dtype=np.int64)
    dst = np.asarray(edge_index[1], dtype=np.int64)
    attr = np.asarray(edge_attr, dtype=np.int64)

    deg = np.bincount(dst, minlength=N).astype(np.float32)
    C = np.zeros((N, R), np.float32)
    np.add.at(C, (dst, attr), 1.0)
    Cdiv = C / np.maximum(deg, 1.0)[:, None]

    win = dst // WSZ  # global window id, 0..127
    order = np.argsort(win, kind="stable")
    src_s, dst_s, attr_s = src[order], dst[order], attr[order]
    wcnt = np.bincount(win, minlength=N // WSZ)
    wstart = np.concatenate([[0], np.cumsum(wcnt)])

    cpw = []
    for w in range(NW):
        mx = 1
        for k in range(NCORES):
            gw = k * NW + w
            mx = max(mx, -(-int(wcnt[gw]) // 128))
        cpw.append(int(mx))
    nch = sum(cpw)

    idx_all, rep_all = [], []
    for k in range(NCORES):
        arr = np.zeros((nch, 128, 2), np.int32)
        arr[:, :, 1] = 255  # padding dstlocal: one-hot never matches
        rep = np.zeros((nch, 256), np.int32)
        rep[:, 0:128] = 255
        ptr = 0
        for w in range(NW):
            gw = k * NW + w
            base = gw * WSZ
            s0, s1 = int(wstart[gw]), int(wstart[gw + 1])
            es, ed, ea = src_s[s0:s1], dst_s[s0:s1], attr_s[s0:s1]
            nreal = s1 - s0
            rows = np.zeros((cpw[w] * 128, 2), np.int32)
            rows[:, 1] = 255
            rows[:nreal, 0] = es
            rows[:nreal, 1] = ed - base
            rrows = np.zeros((cpw[w], 128, 2), np.int32)
            rrows[:, :, :] = rows.reshape(cpw[w], 128, 2)
            arr[ptr : ptr + cpw[w]] = rrows
            rr = np.zeros((cpw[w] * 128, 2), np.int32)
            rr[:, 0] = 255
            rr[:nreal, 0] = ed - base
            rr[:nreal, 1] = ea
            rep[ptr : ptr + cpw[w], 0:128] = rr[:, 0].reshape(cpw[w], 128)
            rep[ptr : ptr + cpw[w], 128:256] = rr[:, 1].reshape(cpw[w], 128)
            ptr += cpw[w]
        idx_all.append(
            np.ascontiguousarray(arr.transpose(1, 0, 2).reshape(128, nch * 2))
        )
        repb = rep.reshape(1, nch * 256).astype(ml_dtypes.bfloat16)
        rep_all.append(np.ascontiguousarray(np.broadcast_to(repb, (128, nch * 256))))
    return cpw, nch, idx_all, rep_all, Cdiv


_cache = {}
LAST_RESULTS = None
LAST_EXEC_NS = None
LAST_RES = None


def kernel(**inputs):
    x = np.asarray(inputs["x"], np.float32)
    rel_emb = np.asarray(inputs["rel_emb"], np.float32)
    w_src = np.asarray(inputs["w_src"], np.float32)
    w_dst = np.asarray(inputs["w_dst"], np.float32)
    w_edge = np.asarray(inputs["w_edge"], np.float32)
    att = np.asarray(inputs["att"], np.float32)
    bias = np.asarray(inputs["bias"], np.float32)
    proj_w = np.asarray(inputs["proj_w"], np.float32)
    proj_b = np.asarray(inputs["proj_b"], np.float32)
    ln_g = np.asarray(inputs["ln_g"], np.float32)
    ln_b = np.asarray(inputs["ln_b"], np.float32)
    edge_index = np.asarray(inputs["edge_index"], np.int32)
    edge_attr = np.asarray(inputs["edge_attr"], np.int32)

    cpw, nch, idx_all, rep_all, Cdiv = _preprocess(edge_index, edge_attr)

    key = (tuple(cpw), nch)
    if key not in _cache:
        _cache[key] = build_program(cpw, nch)
    nc = _cache[key]

    bf = lambda a: np.ascontiguousarray(a).astype(ml_dtypes.bfloat16)
    ws_b = bf(w_src.reshape(L, D, HD))
    wd_b = bf(w_dst.reshape(L, D, HD))
    we_b = bf(w_edge.reshape(L, D, HD))
    att_rep = bf(np.broadcast_to(att.reshape(L, 1, HD), (L, 128, HD)))
    bias_rep = np.ascontiguousarray(
        np.broadcast_to(bias.reshape(L, 1, D), (L, 128, D)), dtype=np.float32
    )
    pw_b = bf(proj_w)
    pb_rep = np.ascontiguousarray(np.broadcast_to(proj_b, (128, D)), np.float32)
    lng_rep = np.ascontiguousarray(np.broadcast_to(ln_g, (128, D)), np.float32)
    lnb_rep = np.ascontiguousarray(np.broadcast_to(ln_b, (128, D)), np.float32)

    in_maps = []
    for k in range(NCORES):
        rows = slice(k * NL, (k + 1) * NL)
        xs = np.ascontiguousarray(x[rows])
        xsT = xs.T.astype(ml_dtypes.bfloat16)  # [768, 2048]
        xT0 = np.ascontiguousarray(
            np.concatenate([xsT[kt * 128 : (kt + 1) * 128, :] for kt in range(6)], axis=1)
        )
        in_maps.append(
            {
                "x_own": xs,
                "xT0": xT0,
                "idx": idx_all[k],
                "rep2": rep_all[k],
                "cdivT": bf(Cdiv[rows].T),
                "w_src": ws_b,
                "w_dst": wd_b,
                "w_edge": we_b,
                "rel_emb": np.ascontiguousarray(rel_emb, np.float32),
                "att_rep": att_rep,
                "bias_rep": bias_rep,
                "proj_w": pw_b,
                "pb_rep": bf(pb_rep),
                "lng_rep": bf(lng_rep),
                "lnb_rep": bf(lnb_rep),
            }
        )

    trace = os.environ.get("GAT_TRACE", "0") == "1"
    res = run_bass_kernel_spmd(
        nc, in_maps, core_ids=list(range(NCORES)), trace=trace
    )
    global LAST_RESULTS, LAST_EXEC_NS, LAST_RES
    LAST_RESULTS = res.results
    LAST_EXEC_NS = res.exec_time_ns
    LAST_RES = res
    out = np.concatenate([res.results[k]["out"] for k in range(NCORES)], axis=0)
    return out.astype(np.float32)


# revision 15
# speedup vs baseline: 1.1045x; 1.0072x over previous
"""Trainium2 Bass kernel for nn_ExpaModel_73478300500036 (3-layer GATv2-style
GNN message passing, N=16384 nodes, E=49152 edges, D=768, H=4 heads).

Strategy (8 NeuronCores, SPMD, dst-sharded):
  - core k owns nodes [k*2048, (k+1)*2048) and computes their output rows.
  - Per layer: hs = x @ Wsrc computed for the own shard, AllGathered to all
    cores (the only collective); hd, he stay local.
  - Real edges grouped by 128-node dst windows, 128 edges/chunk.  Per chunk:
    one indirect row-gather of hs[src]; hd[dst] and he[attr] come from
    one-hot permutation matmuls on the PE (no DMA gathers):
      u = hsg + eqT_dst.T @ hd_win + eqT_attr.T @ he_table
    z = leaky(u), logits via per-head multiply+reduce, p = exp(logits)
    (segment max skipped: logits are O(10), exp is safe in fp32 and
    softmax is shift-invariant).
  - Scatter via one-hot matmuls: po += eq.T @ (p_h * hsg), den via rhs=p.
  - Self-loops (attr = mean of incoming) handled densely per window:
    z_self = leaky(hs_win + hd_win + Cdiv @ he_table); their scatter
    contribution enters the same PSUM groups via lhsT=identity.
  - PSUM: a single rotating pair of [128,2048] fp32 slots (4 banks each)
    serves weight matmuls, permute accumulation, and scatter+den.
  - Projection + LayerNorm + gelu data-parallel over nodes.

Storage dtype bf16, fp32 accumulation.
"""

import os
import sys

sys.path.insert(0, "/opt/trn_rl_repo")

# The RDH collective algorithm (1-56MB messages) crashes the device in this
# environment; force mesh/ring instead.
os.environ.setdefault("NEURON_RT_DBG_RDH_CC", "0")

import ml_dtypes
import numpy as np

import concourse.bass as bass
import concourse.mybir as mybir
import concourse.tile as tile
from concourse.bass_utils import run_bass_kernel_spmd
from concourse.masks import make_identity
from concourse.vector_clock import ScopedClock

# ----------------------------------------------------------------------------
# Workaround: this container's walrus build supports at most ONE sync wait per
# instruction. (a) Tile's tail drain carries several waits -> emit them as
# separate SP EventSemaphore waits; (b) post-pass splits any remaining
# multi-wait instruction.
# ----------------------------------------------------------------------------


def _patched_drain_and_barrier(self, tick_clock, wait_clock):
    nc = self.nc
    probe = mybir.InstDrain(
        name=nc.get_next_instruction_name(), ins=[], outs=[], bass_is_fusable=False
    )
    probe.engine = mybir.EngineType.SP
    wait_clock.add_sem_waits(probe, ScopedClock({None: tick_clock.global_clock}))
    waits = []
    si = probe.sync_info
    if si is not None and si.on_wait:
        waits = list(si.on_wait)
    id2sem = {h.num: h for h in self.sems.allocated().values()}
    for w in waits:
        sem = id2sem.get(w.id)
        assert sem is not None, f"drain wait on unknown sem id {w.id}"
        nc.sync.wait_ge(sem, w.wait_value)
    nc.sync.drain()
    nc.all_engine_barrier()
    assert self.sems is not None
    popped = nc._tile_sem_poison_stack.pop()
    assert popped is self._sem_poison
    nc.clear_and_free_semaphores(list(self.sems.allocated().values()))
    nc.all_engine_barrier()


tile.TileContext._drain_and_barrier = _patched_drain_and_barrier

_split_n = [0]


def _split_multi_waits(nc):
    for f in nc.m.functions:
        for bb in f.blocks:
            insts = list(bb.instructions)
            changed = False
            new_list = []
            for inst in insts:
                si = inst.sync_info
                waits = list(si.on_wait) if (si is not None and si.on_wait) else []
                if len(waits) > 1:
                    changed = True
                    for w in waits[:-1]:
                        _split_n[0] += 1
                        ev = mybir.InstEventSemaphore(
                            name=f"evsplit-{_split_n[0]}", ins=[], outs=[]
                        )
                        ev.engine = inst.engine
                        ev.sync_info = mybir.SyncInfo(on_wait=[w], on_update=[])
                        new_list.append(ev)
                    inst.sync_info = mybir.SyncInfo(
                        on_wait=[waits[-1]],
                        on_update=list(si.on_update) if si.on_update else [],
                    )
                new_list.append(inst)
            if changed:
                bb.instructions = new_list


# ----------------------------------------------------------------------------
# Problem constants (hardcoded per spec)
# ----------------------------------------------------------------------------
NCORES = 8
N = 16384
E = 49152
D = 768
H = 4
L = 3
R = 64
HD = H * D  # 3072
HH = HD // 2  # 1536 (head-pair half)
NL = N // NCORES  # 2048
NW = NL // 128  # 16 windows per core
WSZ = 128
LN_EPS = 1e-5
NEG_SLOPE = 0.2

F32 = mybir.dt.float32
BF16 = mybir.dt.bfloat16
I32 = mybir.dt.int32

AF = mybir.ActivationFunctionType
ALU = mybir.AluOpType


def build_program(cpw, nch):
    nc = bass.Bass("TRN2", num_devices=NCORES)
    maxc = max(cpw)

    # ---- I/O ----
    x_in = nc.dram_tensor("x_own", [NL, D], F32, kind="ExternalInput")
    xT0_in = nc.dram_tensor("xT0", [128, 6 * NL], BF16, kind="ExternalInput")
    idx_in = nc.dram_tensor("idx", [128, nch * 2], I32, kind="ExternalInput")
    rep_in = nc.dram_tensor("rep2", [128, nch * 256], BF16, kind="ExternalInput")
    cdt_in = nc.dram_tensor("cdivT", [R, NL], BF16, kind="ExternalInput")
    ws_in = nc.dram_tensor("w_src", [L, D, HD], BF16, kind="ExternalInput")
    wd_in = nc.dram_tensor("w_dst", [L, D, HD], BF16, kind="ExternalInput")
    we_in = nc.dram_tensor("w_edge", [L, D, HD], BF16, kind="ExternalInput")
    rel_in = nc.dram_tensor("rel_emb", [R, D], F32, kind="ExternalInput")
    att_in = nc.dram_tensor("att_rep", [L, 128, HD], BF16, kind="ExternalInput")
    bias_in = nc.dram_tensor("bias_rep", [L, 128, D], F32, kind="ExternalInput")
    pw_in = nc.dram_tensor("proj_w", [D, D], BF16, kind="ExternalInput")
    pb_in = nc.dram_tensor("pb_rep", [128, D], BF16, kind="ExternalInput")
    lng_in = nc.dram_tensor("lng_rep", [128, D], BF16, kind="ExternalInput")
    lnb_in = nc.dram_tensor("lnb_rep", [128, D], BF16, kind="ExternalInput")
    out_t = nc.dram_tensor("out", [NL, D], F32, kind="ExternalOutput")

    # ---- internal DRAM ----
    ag_in = nc.dram_tensor("ag_in", [NL, HD], BF16, kind="Internal")
    hs_full = nc.dram_tensor(
        "hs_full", [N, HD], BF16, kind="Internal", addr_space="Shared"
    )
    hd_dram = nc.dram_tensor("hd_dram", [NL, HD], BF16, kind="Internal")
    xb = [nc.dram_tensor(f"xb{i}", [NL, D], F32, kind="Internal") for i in range(2)]
    xbf = [nc.dram_tensor(f"xbf{i}", [NL, D], BF16, kind="Internal") for i in range(2)]

    with tile.TileContext(nc) as tc:
        with (
            tc.tile_pool(name="sb", bufs=1) as sp,
            tc.tile_pool(name="ps", bufs=1, space="PSUM") as pp,
        ):
            # ---- static tiles ----
            ident = sp.tile([128, 128], F32, tag="ident")
            make_identity(nc, ident[:])
            identb = sp.tile([128, 128], BF16, tag="identb")
            nc.vector.tensor_copy(out=identb[:], in_=ident[:])
            ioti = sp.tile([128, 128], mybir.dt.int16, tag="ioti")
            nc.gpsimd.iota(ioti[:], pattern=[[1, 128]], base=0, channel_multiplier=0)
            iotab = sp.tile([128, 128], BF16, tag="iotab")
            nc.vector.tensor_copy(out=iotab[:], in_=ioti[:])
            ioPi = sp.tile([128, 1], I32, tag="ioPi")
            nc.gpsimd.iota(ioPi[:], pattern=[[0, 1]], base=0, channel_multiplier=1)
            ioPb = sp.tile([128, 1], BF16, tag="ioPb")
            nc.vector.tensor_copy(out=ioPb[:], in_=ioPi[:])
            ones_b = sp.tile([128, 1], BF16, tag="ones")
            nc.vector.memset(ones_b[:], 1.0)

            idx_t = sp.tile([128, nch * 2], I32, tag="idx")
            nc.sync.dma_start(out=idx_t[:], in_=idx_in[:])
            idx2 = idx_t[:].rearrange("p (c f) -> p c f", f=2)

            cdt = sp.tile([R, NL], BF16, tag="cdt")
            nc.sync.dma_start(out=cdt[:], in_=cdt_in[:])
            rel_sb = sp.tile([R, D], F32, tag="rel")
            nc.sync.dma_start(out=rel_sb[:], in_=rel_in[:])

            # relT [768, 64] as 6 blocks [128, 64], via PE transpose (fp32)
            relT = sp.tile([128, 6 * R], BF16, tag="relT")
            for kt in range(6):
                pt = pp.tile([128, 2048], F32, tag="psA", bufs=2)
                nc.tensor.transpose(
                    out=pt[:, 0:R],
                    in_=rel_sb[:, kt * 128 : (kt + 1) * 128],
                    identity=ident[:R, :R],
                )
                nc.scalar.copy(out=relT[:, kt * R : (kt + 1) * R], in_=pt[:, 0:R])

            pbb = sp.tile([128, D], BF16, tag="pbb")
            nc.sync.dma_start(out=pbb[:], in_=pb_in[:])
            lngb = sp.tile([128, D], BF16, tag="lngb")
            nc.sync.dma_start(out=lngb[:], in_=lng_in[:])
            lnbb = sp.tile([128, D], BF16, tag="lnbb")
            nc.sync.dma_start(out=lnbb[:], in_=lnb_in[:])

            het = sp.tile([R, HD], BF16, tag="het")
            pself_all = sp.tile([128, 4 * NW], F32, tag="pself")

            def wload(w_dram, l, fh):
                wt = sp.tile([128, 6 * 1536], BF16, tag="wt", bufs=2)
                for kt in range(6):
                    nc.sync.dma_start(
                        out=wt[:, kt * 1536 : (kt + 1) * 1536],
                        in_=w_dram[
                            l, kt * 128 : (kt + 1) * 128, fh * 1536 : (fh + 1) * 1536
                        ],
                    )
                return wt

            def proj_half(xT, wt, fh, dest):
                """dest[:, fh half] <- x @ W (one 1536-col half)."""
                for m in range(NW):
                    ps = pp.tile([128, 2048], F32, tag="psA", bufs=2)
                    for kt in range(6):
                        for s in range(3):
                            nc.tensor.matmul(
                                ps[:, s * 512 : (s + 1) * 512],
                                lhsT=xT[:, kt * NL + m * 128 : kt * NL + (m + 1) * 128],
                                rhs=wt[:, kt * 1536 + s * 512 : kt * 1536 + (s + 1) * 512],
                                start=(kt == 0),
                                stop=(kt == 5),
                            )
                    ev = sp.tile([128, 1536], BF16, tag="stage", bufs=2)
                    nc.scalar.copy(out=ev[:], in_=ps[:, 0:1536])
                    nc.sync.dma_start(
                        out=dest[m * 128 : (m + 1) * 128, fh * 1536 : (fh + 1) * 1536],
                        in_=ev[:],
                    )

            x_cur = x_in
            xbf_prev = None
            for l in range(L):
                # ---- xT ----
                xT = sp.tile([128, 6 * NL], BF16, tag="xT")
                if l == 0:
                    nc.sync.dma_start(out=xT[:], in_=xT0_in[:])
                else:
                    for kt in range(6):
                        nc.sync.dma_start_transpose(
                            out=xT[:, kt * NL : (kt + 1) * NL],
                            in_=xbf_prev[:, kt * 128 : (kt + 1) * 128],
                        )

                att_t = sp.tile([128, HD], BF16, tag="att", bufs=1)
                nc.sync.dma_start(out=att_t[:], in_=att_in[l])
                bias_b = sp.tile([128, D], F32, tag="biasb", bufs=1)
                nc.sync.dma_start(out=bias_b[:], in_=bias_in[l])

                # ---- hs -> ag_in, then AllGather ----
                for fh in range(2):
                    wt = wload(ws_in, l, fh)
                    proj_half(xT, wt, fh, ag_in)
                # ---- hd -> hd_dram (overlaps AllGather) ----
                for fh in range(2):
                    wt = wload(wd_in, l, fh)
                    proj_half(xT, wt, fh, hd_dram)

                # ---- he table: het = rel_emb @ Wedge  [64, 3072] ----
                for fh in range(2):
                    wt = wload(we_in, l, fh)
                    ps = pp.tile([128, 2048], F32, tag="psA", bufs=2)
                    for kt in range(6):
                        for s in range(3):
                            nc.tensor.matmul(
                                ps[:R, s * 512 : (s + 1) * 512],
                                lhsT=relT[:, kt * R : (kt + 1) * R],
                                rhs=wt[:, kt * 1536 + s * 512 : kt * 1536 + (s + 1) * 512],
                                start=(kt == 0),
                                stop=(kt == 5),
                            )
                    nc.scalar.copy(
                        out=het[:, fh * 1536 : (fh + 1) * 1536], in_=ps[:R, 0:1536]
                    )

                # ---- self-loop pass per window (overlaps AllGather) ----
                for w in range(NW):
                    rows = slice(w * 128, (w + 1) * 128)
                    slog = sp.tile([128, H], F32, tag="slog", bufs=2)
                    for hp in range(2):
                        csl = slice(hp * 1536, (hp + 1) * 1536)
                        hsw = sp.tile([128, 1536], BF16, tag="hswA", bufs=2)
                        nc.sync.dma_start(out=hsw[:], in_=ag_in[rows, csl])
                        hdw = sp.tile([128, 1536], BF16, tag="hdwA", bufs=2)
                        nc.sync.dma_start(out=hdw[:], in_=hd_dram[rows, csl])
                        ps = pp.tile([128, 2048], F32, tag="psA", bufs=2)
                        for s in range(3):
                            nc.tensor.matmul(
                                ps[:, s * 512 : (s + 1) * 512],
                                lhsT=cdt[:, w * 128 : (w + 1) * 128],
                                rhs=het[:, hp * 1536 + s * 512 : hp * 1536 + (s + 1) * 512],
                                start=True,
                                stop=True,
                            )
                        hl = sp.tile([128, 1536], BF16, tag="uhe", bufs=3)
                        nc.scalar.copy(out=hl[:], in_=ps[:, 0:1536])
                        nc.vector.tensor_add(out=hl[:], in0=hl[:], in1=hsw[:])
                        nc.vector.tensor_add(out=hl[:], in0=hl[:], in1=hdw[:])
                        z = sp.tile([128, 1536], BF16, tag="z", bufs=2)
                        nc.scalar.activation(
                            out=z[:], in_=hl[:], func=AF.Prelu, alpha=NEG_SLOPE
                        )
                        sc = sp.tile([128, D], BF16, tag="za", bufs=1)
                        for hh in range(2):
                            h = hp * 2 + hh
                            nc.vector.scalar_tensor_tensor(
                                out=sc[:],
                                in0=z[:, hh * D : (hh + 1) * D],
                                scalar=1.0,
                                in1=att_t[:, h * D : (h + 1) * D],
                                op0=ALU.mult,
                                op1=ALU.mult,
                                accum_out=slog[:, h : h + 1],
                            )
                    nc.scalar.activation(
                        out=pself_all[:, 4 * w : 4 * w + 4], in_=slog[:], func=AF.Exp
                    )

                nc.gpsimd.collective_compute(
                    "AllGather",
                    ALU.bypass,
                    ins=[ag_in[:]],
                    outs=[hs_full[:]],
                    replica_groups=[list(range(NCORES))],
                )

                # ---- edge phase ----
                ci = 0
                for w in range(NW):
                    rows = slice(w * 128, (w + 1) * 128)
                    nch_w = cpw[w]
                    rep_t = sp.tile([128, maxc * 256], BF16, tag="rep", bufs=1)
                    nc.sync.dma_start(
                        out=rep_t[:, 0 : nch_w * 256],
                        in_=rep_in[:, ci * 256 : (ci + nch_w) * 256],
                    )
                    hdw2 = sp.tile([128, HD], BF16, tag="hdwB", bufs=2)
                    nc.sync.dma_start(out=hdw2[:], in_=hd_dram[rows, :])
                    hsw2 = sp.tile([128, HD], BF16, tag="hswB", bufs=2)
                    nc.sync.dma_start(out=hsw2[:], in_=ag_in[rows, :])
                    xc = sp.tile([128, D], F32, tag="xc", bufs=1)
                    nc.sync.dma_start(out=xc[:], in_=x_cur[rows, :])

                    hsgs = []
                    eqs = []
                    ps_list = []
                    logit = sp.tile([128, H * maxc], F32, tag="logit", bufs=2)
                    for c in range(nch_w):
                        cidx = ci + c
                        hsg = sp.tile([128, HD], BF16, tag="hsg", bufs=maxc + 1)
                        nc.gpsimd.indirect_dma_start(
                            out=hsg[:],
                            out_offset=None,
                            in_=hs_full[:, :],
                            in_offset=bass.IndirectOffsetOnAxis(
                                ap=idx2[:, cidx, 0:1], axis=0
                            ),
                        )
                        hsgs.append(hsg)
                        dstf = sp.tile([128, 1], BF16, tag="dstf", bufs=3)
                        nc.vector.tensor_copy(out=dstf[:], in_=idx2[:, cidx, 1:2])
                        eq = sp.tile([128, 128], BF16, tag="eq", bufs=maxc + 2)
                        nc.vector.tensor_tensor(
                            out=eq[:],
                            in0=dstf[:].to_broadcast([128, 128]),
                            in1=iotab[:],
                            op=ALU.is_equal,
                        )
                        eqs.append(eq)
                        eqT = sp.tile([128, 128], BF16, tag="eqT", bufs=3)
                        nc.vector.tensor_tensor(
                            out=eqT[:],
                            in0=ioPb[:].to_broadcast([128, 128]),
                            in1=rep_t[:, c * 256 : c * 256 + 128],
                            op=ALU.is_equal,
                        )
                        eqh = sp.tile([128, 128], BF16, tag="eqh", bufs=3)
                        nc.vector.tensor_tensor(
                            out=eqh[:R, :],
                            in0=ioPb[:R, :].to_broadcast([R, 128]),
                            in1=rep_t[:R, c * 256 + 128 : c * 256 + 256],
                            op=ALU.is_equal,
                        )
                        for hp in range(2):
                            ps = pp.tile([128, 2048], F32, tag="psA", bufs=2)
                            for s in range(3):
                                nc.tensor.matmul(
                                    ps[:, s * 512 : (s + 1) * 512],
                                    lhsT=eqT[:],
                                    rhs=hdw2[
                                        :, hp * 1536 + s * 512 : hp * 1536 + (s + 1) * 512
                                    ],
                                    start=True,
                                    stop=False,
                                )
                            for s in range(3):
                                nc.tensor.matmul(
                                    ps[:, s * 512 : (s + 1) * 512],
                                    lhsT=eqh[:R, :],
                                    rhs=het[
                                        :, hp * 1536 + s * 512 : hp * 1536 + (s + 1) * 512
                                    ],
                                    start=False,
                                    stop=True,
                                )
                            uhe = sp.tile([128, 1536], BF16, tag="uhe", bufs=3)
                            nc.scalar.copy(out=uhe[:], in_=ps[:, 0:1536])
                            half = slice(hp * 1536, (hp + 1) * 1536)
                            nc.vector.tensor_add(
                                out=uhe[:], in0=uhe[:], in1=hsg[:, half]
                            )
                            z = sp.tile([128, 1536], BF16, tag="z", bufs=2)
                            nc.scalar.activation(
                                out=z[:], in_=uhe[:], func=AF.Prelu, alpha=NEG_SLOPE
                            )
                            sc = sp.tile([128, D], BF16, tag="za", bufs=1)
                            for hh in range(2):
                                h = hp * 2 + hh
                                nc.vector.scalar_tensor_tensor(
                                    out=sc[:],
                                    in0=z[:, hh * D : (hh + 1) * D],
                                    scalar=1.0,
                                    in1=att_t[:, h * D : (h + 1) * D],
                                    op0=ALU.mult,
                                    op1=ALU.mult,
                                    accum_out=logit[:, c * H + h : c * H + h + 1],
                                )

                    # p = exp(logits), bf16 copy for den rhs
                    p_t = sp.tile([128, H * maxc], F32, tag="pt", bufs=2)
                    pb_t = sp.tile([128, H * maxc], BF16, tag="ptb", bufs=2)
                    nc.scalar.activation(
                        out=p_t[:, 0 : H * nch_w],
                        in_=logit[:, 0 : H * nch_w],
                        func=AF.Exp,
                    )
                    nc.vector.tensor_copy(
                        out=pb_t[:, 0 : H * nch_w], in_=p_t[:, 0 : H * nch_w]
                    )
                    psb = sp.tile([128, H], BF16, tag="psb", bufs=2)
                    nc.vector.tensor_copy(
                        out=psb[:], in_=pself_all[:, 4 * w : 4 * w + 4]
                    )

                    # ---- scatter + finalize (per head-pair pass) ----
                    acc = sp.tile([128, D], F32, tag="acc", bufs=2)
                    for hp in range(2):
                        po = pp.tile([128, 2048], F32, tag="psA", bufs=2)
                        nchunks = nch_w + 1  # + self-loop diag
                        for c in range(nchunks):
                            last = c == nchunks - 1
                            if last:
                                # self-loop: v = pself_h * hs_win, lhsT = I
                                v = sp.tile([128, 1536], BF16, tag="v", bufs=2)
                                for hh in range(2):
                                    h = hp * 2 + hh
                                    nc.vector.tensor_scalar_mul(
                                        v[:, hh * D : (hh + 1) * D],
                                        in0=hsw2[:, h * D : (h + 1) * D],
                                        scalar1=pself_all[:, 4 * w + h : 4 * w + h + 1],
                                    )
                                lhsT = identb[:]
                                pcols = psb[:, hp * 2 : hp * 2 + 2]
                            else:
                                v = sp.tile([128, 1536], BF16, tag="v", bufs=2)
                                for hh in range(2):
                                    h = hp * 2 + hh
                                    nc.vector.tensor_scalar_mul(
                                        v[:, hh * D : (hh + 1) * D],
                                        in0=hsgs[c][:, (hp * 2 + hh) * D : (hp * 2 + hh + 1) * D],
                                        scalar1=p_t[:, c * H + h : c * H + h + 1],
                                    )
                                lhsT = eqs[c][:]
                                pcols = pb_t[:, c * H + hp * 2 : c * H + hp * 2 + 2]
                            for s in range(3):
                                nc.tensor.matmul(
                                    po[:, s * 512 : (s + 1) * 512],
                                    lhsT=lhsT,
                                    rhs=v[:, s * 512 : (s + 1) * 512],
                                    start=(c == 0),
                                    stop=last,
                                )
                            nc.tensor.matmul(
                                po[:, 1536:1538],
                                lhsT=lhsT,
                                rhs=pcols,
                                start=(c == 0),
                                stop=last,
                            )
                        rden = sp.tile([128, 2], F32, tag="rden", bufs=2)
                        nc.vector.reciprocal(out=rden[:], in_=po[:, 1536:1538])
                        nc.vector.tensor_scalar_mul(
                            rden[:], in0=rden[:], scalar1=1.0 / H
                        )
                        if hp == 0:
                            nc.scalar.activation(
                                out=acc[:],
                                in_=po[:, 0:D],
                                func=AF.Copy,
                                scale=rden[:, 0:1],
                            )
                        else:
                            nc.vector.scalar_tensor_tensor(
                                out=acc[:],
                                in0=po[:, 0:D],
                                scalar=rden[:, 0:1],
                                in1=acc[:],
                                op0=ALU.mult,
                                op1=ALU.add,
                            )
                        nc.vector.scalar_tensor_tensor(
                            out=acc[:],
                            in0=po[:, D : 2 * D],
                            scalar=rden[:, 1:2],
                            in1=acc[:],
                            op0=ALU.mult,
                            op1=ALU.add,
                        )

                    # ---- final per window ----
                    nc.vector.tensor_add(out=acc[:], in0=acc[:], in1=bias_b[:])
                    g_t = sp.tile([128, D], F32, tag="fin", bufs=2)
                    nc.scalar.activation(out=g_t[:], in_=acc[:], func=AF.Gelu_apprx_tanh)
                    nc.vector.tensor_add(out=g_t[:], in0=g_t[:], in1=xc[:])
                    nc.sync.dma_start(out=xb[l % 2][rows, :], in_=g_t[:])
                    gb = sp.tile([128, D], BF16, tag="finb", bufs=2)
                    nc.vector.tensor_copy(out=gb[:], in_=g_t[:])
                    nc.sync.dma_start(out=xbf[l % 2][rows, :], in_=gb[:])
                    ci += nch_w

                x_cur = xb[l % 2]
                xbf_prev = xbf[l % 2]

            # ================= projection + LayerNorm + gelu =================
            xT = sp.tile([128, 6 * NL], BF16, tag="xT")
            for kt in range(6):
                nc.sync.dma_start_transpose(
                    out=xT[:, kt * NL : (kt + 1) * NL],
                    in_=xbf_prev[:, kt * 128 : (kt + 1) * 128],
                )
            pwt = sp.tile([128, 6 * D], BF16, tag="wt", bufs=2)
            for kt in range(6):
                nc.sync.dma_start(
                    out=pwt[:, kt * D : (kt + 1) * D],
                    in_=pw_in[kt * 128 : (kt + 1) * 128, :],
                )
            for m in range(NW):
                rows = slice(m * 128, (m + 1) * 128)
                ps = pp.tile([128, 2048], F32, tag="psA", bufs=2)
                for kt in range(6):
                    for a, b in ((0, 512), (512, 768)):
                        nc.tensor.matmul(
                            ps[:, a:b],
                            lhsT=xT[:, kt * NL + m * 128 : kt * NL + (m + 1) * 128],
                            rhs=pwt[:, kt * D + a : kt * D + b],
                            start=(kt == 0),
                            stop=(kt == 5),
                        )
                y0 = sp.tile([128, D], F32, tag="acc", bufs=2)
                nc.vector.tensor_add(out=y0[:], in0=ps[:, 0:D], in1=pbb[:])
                mu = sp.tile([128, 1], F32, tag="stats", bufs=4)
                nc.vector.tensor_reduce(
                    out=mu[:], in_=y0[:], axis=mybir.AxisListType.X, op=ALU.add
                )
                nc.vector.tensor_scalar_mul(mu[:], in0=mu[:], scalar1=1.0 / D)
                xc2 = sp.tile([128, D], F32, tag="fin", bufs=2)
                nc.vector.tensor_scalar_sub(xc2[:], in0=y0[:], scalar1=mu[:])
                var = sp.tile([128, 1], F32, tag="stats", bufs=4)
                nc.vector.scalar_tensor_tensor(
                    out=y0[:],
                    in0=xc2[:],
                    scalar=1.0,
                    in1=xc2[:],
                    op0=ALU.mult,
                    op1=ALU.mult,
                    accum_out=var[:],
                )
                nc.vector.tensor_scalar(
                    var[:], in0=var[:], scalar1=1.0 / D, scalar2=LN_EPS,
                    op0=ALU.mult, op1=ALU.add,
                )
                sd = sp.tile([128, 1], F32, tag="stats", bufs=4)
                nc.scalar.activation(out=sd[:], in_=var[:], func=AF.Sqrt)
                rstd = sp.tile([128, 1], F32, tag="stats", bufs=4)
                nc.vector.reciprocal(out=rstd[:], in_=sd[:])
                nc.vector.tensor_scalar_mul(y0[:], in0=xc2[:], scalar1=rstd[:])
                nc.vector.tensor_mul(out=y0[:], in0=y0[:], in1=lngb[:])
                nc.vector.tensor_add(out=y0[:], in0=y0[:], in1=lnbb[:])
                og = sp.tile([128, D], F32, tag="fin", bufs=2)
                nc.scalar.activation(out=og[:], in_=y0[:], func=AF.Gelu_apprx_tanh)
                nc.sync.dma_start(out=out_t[rows, :], in_=og[:])

    _split_multi_waits(nc)
    return nc


# ----------------------------------------------------------------------------
# Host side
# ----------------------------------------------------------------------------


def _preprocess(edge_index, edge_attr):
    src = np.asarray(edge_index[0], dtype=np.int64)
    dst = np.asarray(edge_index[1], dtype=np.int64)
    attr = np.asarray(edge_attr, dtype=np.int64)

    deg = np.bincount(dst, minlength=N).astype(np.float32)
    C = np.zeros((N, R), np.float32)
    np.add.at(C, (dst, attr), 1.0)
    Cdiv = C / np.maximum(deg, 1.0)[:, None]

    win = dst // WSZ  # global window id, 0..127
    order = np.argsort(win, kind="stable")
    src_s, dst_s, attr_s = src[order], dst[order], attr[order]
    wcnt = np.bincount(win, minlength=N // WSZ)
    wstart = np.concatenate([[0], np.cumsum(wcnt)])

    cpw = []
    for w in range(NW):
        mx = 1
        for k in range(NCORES):
            gw = k * NW + w
            mx = max(mx, -(-int(wcnt[gw]) // 128))
        cpw.append(int(mx))
    nch = sum(cpw)

    idx_all, rep_all = [], []
    for k in range(NCORES):
        arr = np.zeros((nch, 128, 2), np.int32)
        arr[:, :, 1] = 255  # padding dstlocal: one-hot never matches
        rep = np.zeros((nch, 256), np.int32)
        rep[:, 0:128] = 255
        ptr = 0
        for w in range(NW):
            gw = k * NW + w
            base = gw * WSZ
            s0, s1 = int(wstart[gw]), int(wstart[gw + 1])
            es, ed, ea = src_s[s0:s1], dst_s[s0:s1], attr_s[s0:s1]
            nreal = s1 - s0
            rows = np.zeros((cpw[w] * 128, 2), np.int32)
            rows[:, 1] = 255
            rows[:nreal, 0] = es
            rows[:nreal, 1] = ed - base
            rrows = np.zeros((cpw[w], 128, 2), np.int32)
            rrows[:, :, :] = rows.reshape(cpw[w], 128, 2)
            arr[ptr : ptr + cpw[w]] = rrows
            rr = np.zeros((cpw[w] * 128, 2), np.int32)
            rr[:, 0] = 255
            rr[:nreal, 0] = ed - base
            rr[:nreal, 1] = ea
            rep[ptr : ptr + cpw[w], 0:128] = rr[:, 0].reshape(cpw[w], 128)
            rep[ptr : ptr + cpw[w], 128:256] = rr[:, 1].reshape(cpw[w], 128)
            ptr += cpw[w]
        idx_all.append(
            np.ascontiguousarray(arr.transpose(1, 0, 2).reshape(128, nch * 2))
        )
        repb = rep.reshape(1, nch * 256).astype(ml_dtypes.bfloat16)
        rep_all.append(np.ascontiguousarray(np.broadcast_to(repb, (128, nch * 256))))
    return cpw, nch, idx_all, rep_all, Cdiv


_cache = {}
LAST_RESULTS = None
LAST_EXEC_NS = None
LAST_RES = None


def kernel(**inputs):
    x = np.asarray(inputs["x"], np.float32)
    rel_emb = np.asarray(inputs["rel_emb"], np.float32)
    w_src = np.asarray(inputs["w_src"], np.float32)
    w_dst = np.asarray(inputs["w_dst"], np.float32)
    w_edge = np.asarray(inputs["w_edge"], np.float32)
    att = np.asarray(inputs["att"], np.float32)
    bias = np.asarray(inputs["bias"], np.float32)
    proj_w = np.asarray(inputs["proj_w"], np.float32)
    proj_b = np.asarray(inputs["proj_b"], np.float32)
    ln_g = np.asarray(inputs["ln_g"], np.float32)
    ln_b = np.asarray(inputs["ln_b"], np.float32)
    edge_index = np.asarray(inputs["edge_index"], np.int32)
    edge_attr = np.asarray(inputs["edge_attr"], np.int32)

    cpw, nch, idx_all, rep_all, Cdiv = _preprocess(edge_index, edge_attr)

    key = (tuple(cpw), nch)
    if key not in _cache:
        _cache[key] = build_program(cpw, nch)
    nc = _cache[key]

    bf = lambda a: np.ascontiguousarray(a).astype(ml_dtypes.bfloat16)
    ws_b = bf(w_src.reshape(L, D, HD))
    wd_b = bf(w_dst.reshape(L, D, HD))
    we_b = bf(w_edge.reshape(L, D, HD))
    att_rep = bf(np.broadcast_to(att.reshape(L, 1, HD), (L, 128, HD)))
    bias_rep = np.ascontiguousarray(
        np.broadcast_to(bias.reshape(L, 1, D), (L, 128, D)), dtype=np.float32
    )
    pw_b = bf(proj_w)
    pb_rep = np.ascontiguousarray(np.broadcast_to(proj_b, (128, D)), np.float32)
    lng_rep = np.ascontiguousarray(np.broadcast_to(ln_g, (128, D)), np.float32)
    lnb_rep = np.ascontiguousarray(np.broadcast_to(ln_b, (128, D)), np.float32)

    in_maps = []
    for k in range(NCORES):
        rows = slice(k * NL, (k + 1) * NL)
        xs = np.ascontiguousarray(x[rows])
        xsT = xs.T.astype(ml_dtypes.bfloat16)  # [768, 2048]
        xT0 = np.ascontiguousarray(
            np.concatenate([xsT[kt * 128 : (kt + 1) * 128, :] for kt in range(6)], axis=1)
        )
        in_maps.append(
            {
                "x_own": xs,
                "xT0": xT0,
                "idx": idx_all[k],
                "rep2": rep_all[k],
                "cdivT": bf(Cdiv[rows].T),
                "w_src": ws_b,
                "w_dst": wd_b,
                "w_edge": we_b,
                "rel_emb": np.ascontiguousarray(rel_emb, np.float32),
                "att_rep": att_rep,
                "bias_rep": bias_rep,
                "proj_w": pw_b,
                "pb_rep": bf(pb_rep),
                "lng_rep": bf(lng_rep),
                "lnb_rep": bf(lnb_rep),
            }
        )

    trace = os.environ.get("GAT_TRACE", "0") == "1"
    res = run_bass_kernel_spmd(
        nc, in_maps, core_ids=list(range(NCORES)), trace=trace
    )
    global LAST_RESULTS, LAST_EXEC_NS, LAST_RES
    LAST_RESULTS = res.results
    LAST_EXEC_NS = res.exec_time_ns
    LAST_RES = res
    out = np.concatenate([res.results[k]["out"] for k in range(NCORES)], axis=0)
    return out.astype(np.float32)
